# revision 14
# baseline (speedup 1.0000x reference)
"""Multi-head attention on 8 Trainium2 NeuronCores (Bass/Tile), fp8 edition.

Problem: x[2,2048,1024] -> qkv proj (16 heads, hd=64) -> softmax(QK^T/8)V
-> out proj.  mask is all-ones (per spec) and is ignored.

Sharding: core c owns heads {2c, 2c+1} for BOTH batches (tensor-parallel
QKV + attention).  An 8-core AllToAll converts the head-sharded attention
output into a sequence-sharded full-feature activation; core c ends up
with global row chunk c (batch c//4, rows (c%4)*512..) and computes the
output projection full-width.

All matmuls run in fp8e4m3 with DoubleRow perf mode (2 K-subtiles per
pass).  Scales: weights x32 into fp8 range; scores accumulate f32 as
8192*score_true; exp applies 1/8192.  Biases:
  - k bias dropped (softmax-invariant along keys),
  - q bias folded into scores via an augmented 33rd contraction row
    (k'' row 32 holds fp8(bq.k)/4, q'' row 32 holds 4.0),
  - v bias folded into the output-projection bias (b_eff = b_out + W_o@b_v),
  - b_eff enters the out-proj PSUM via an fp8 ones-chunk matmul.
exp is computed per key-block [128,1024] and split between the Act engine
(table Exp, fp8 out) and the DVE (bit-exact int8 exp2 trick: fp8 bit
pattern = round(score*8*log2e/8192 + 55.656), verified vs HW).

PSUM: scores [128,1024] x2 (4 banks) + AV [65,1024] (2) + work [*,512]
x2 (2) = 8 banks exactly.
"""

import numpy as np
from contextlib import ExitStack

import concourse.bass as bass
import concourse.mybir as mybir
import concourse.tile as tile
from concourse import bacc
from concourse.bass_utils import run_bass_kernel_spmd

BF16 = mybir.dt.bfloat16
F32 = mybir.dt.float32
FP8 = mybir.dt.float8e4
I8 = mybir.dt.int8
NPFP8 = mybir.dt.np(FP8)
DRM = mybir.MatmulPerfMode.DoubleRow
Exp = mybir.ActivationFunctionType.Exp

D, H, HD, B, S = 1024, 16, 64, 2, 2048
NCORES = 8
HPC = 2              # heads per core
FPC = HPC * HD       # 128 features per core
SS = B * S           # 4096 stacked sequence (batch-major)
SC = 512             # output rows per core (post all-to-all)
NKB = S // 128       # 16 key blocks per batch
NQC = S // 512       # 4 query chunks per batch
VW = HD + 1          # live v columns per head (vd + ones)
VWP = 80             # padded v block stride (16-aligned)

EXPSC = 1.0 / 8192.0
A_TRICK = 8.0 * np.log2(np.e) / 8192.0
B_TRICK = 56.0 - 0.344   # fp8 exponent offset + PWL centering

# exp engine per kb: 'A' = Act (table exp), 'D' = DVE (int8 trick)
EXP_ENG = "ADAADADAADADAADA"


def _build_nc(with_collective: bool = True):
    nc = bacc.Bacc("TRN2", target_bir_lowering=False, debug=False,
                   num_devices=NCORES)
    xt = nc.dram_tensor("xt", [D, SS], FP8, kind="ExternalInput").ap()
    wqk = nc.dram_tensor("wqk", [D, 2 * FPC], FP8, kind="ExternalInput").ap()
    bq128 = nc.dram_tensor("bq128", [128, 2], FP8, kind="ExternalInput").ap()
    wv = nc.dram_tensor("wv", [D, FPC], FP8, kind="ExternalInput").ap()
    wout = nc.dram_tensor("wout", [D, D], FP8, kind="ExternalInput").ap()
    bout8 = nc.dram_tensor("bout8", [8, D], FP8, kind="ExternalInput").ap()
    onesr = nc.dram_tensor("onesr", [VW, HD], mybir.dt.float32r,
                           kind="ExternalInput").ap()
    out = nc.dram_tensor("out", [SC, D], F32, kind="ExternalOutput").ap()

    with ExitStack() as ctx:
        tc = ctx.enter_context(tile.TileContext(nc))
        persist = ctx.enter_context(tc.tile_pool(name="persist", bufs=1))
        pexp = ctx.enter_context(tc.tile_pool(name="pexp", bufs=6))
        pwork = ctx.enter_context(tc.tile_pool(name="pwork", bufs=3))
        pscore = ctx.enter_context(tc.tile_pool(name="pscore", bufs=3,
                                                space="PSUM"))
        pav = ctx.enter_context(tc.tile_pool(name="pav", bufs=1,
                                             space="PSUM"))
        pps = pscore
        dram = ctx.enter_context(tc.tile_pool(name="dram", bufs=2,
                                              space="DRAM"))

        # ---------------- persistent SBUF ----------------
        xt_sb = [[persist.tile([128, 8 * 512], FP8, tag=f"xt{b}_{cq}",
                               name=f"xt{b}_{cq}") for cq in range(4)]
                 for b in range(B)]
        wqk_sb = persist.tile([128, 8 * 2 * FPC], FP8, tag="wqk",
                              name="wqk_sb")
        bq_sb = persist.tile([128, 2], FP8, tag="bq", name="bq_sb")
        wv_sb = persist.tile([128, 8 * FPC], FP8, tag="wv", name="wv_sb")
        wout_sb = persist.tile([128, 8 * D], FP8, tag="wout", name="wout_sb")
        bout_sb = persist.tile([8, D], FP8, tag="bout", name="bout_sb")
        ones16 = persist.tile([8, 128], FP8, tag="ones16", name="ones16")
        nc.gpsimd.memset(ones16, 0.0625)

        # q2s/k2s[b][h]: [33, 2*2048] — hd-half t at cols t*2048;
        # row 32: q'' ones (4.0 at t0) / k'' bias row (device-filled)
        q2s = [[persist.tile([33, 2 * S], FP8, tag=f"q2_{b}_{h}",
                             name=f"q2_{b}_{h}") for h in range(HPC)]
               for b in range(B)]
        k2s = [[persist.tile([33, 2 * S], FP8, tag=f"k2_{b}_{h}",
                             name=f"k2_{b}_{h}") for h in range(HPC)]
               for b in range(B)]
        for b in range(B):
            for h in range(HPC):
                nc.gpsimd.memset(q2s[b][h][32:33, 0:S], 4.0)
                nc.gpsimd.memset(q2s[b][h][32:33, S:2 * S], 0.0)
                nc.gpsimd.memset(k2s[b][h][32:33, S:2 * S], 0.0)

        # v_all[b]: [128, kb(16) x h(2) x 80]; col 64 = ones
        v_all = [persist.tile([128, NKB * HPC * VWP], FP8, tag=f"v{b}",
                              name=f"v{b}") for b in range(B)]
        for b in range(B):
            vr = v_all[b].rearrange("p (kb h w) -> p kb h w", kb=NKB, h=HPC)
            nc.gpsimd.memset(vr[:, :, :, HD:VW], 1.0)

        att_sb = [[persist.tile([64, S], FP8, tag=f"att{b}_{h}",
                                name=f"att{b}_{h}") for h in range(HPC)]
                  for b in range(B)]
        ones32 = persist.tile([VW, HD], mybir.dt.float32r, tag="ones32",
                              name="ones32")
        ao_sb = persist.tile([128, 8 * SC], FP8, tag="ao", name="ao_sb")
        part_sb = [persist.tile([128, 512], BF16, tag=f"part{g}",
                                name=f"part{g}") for g in range(8)]
        qkstage = [persist.tile([128, 2 * S], FP8, tag=f"qkst{b}",
                                name=f"qkst{b}") for b in range(B)]
        bstage = [persist.tile([2, S], FP8, tag=f"bst{b}", name=f"bst{b}")
                  for b in range(B)]

        # ---------------- loads (ordered by first use) ----------------
        def xt_chunk_ap(b, cq):
            return bass.AP(tensor=xt.tensor, offset=b * S + cq * 512,
                           ap=[[SS, 128], [128 * SS, 8], [1, 512]])

        wqk_src = bass.AP(tensor=wqk.tensor, offset=0,
                          ap=[[2 * FPC, 128], [128 * 2 * FPC, 8], [1, 2 * FPC]])
        nc.sync.dma_start(
            out=wqk_sb.rearrange("p (i f) -> p i f", i=8), in_=wqk_src)
        for cq in range(2):
            nc.sync.dma_start(
                out=xt_sb[0][cq].rearrange("p (i s) -> p i s", i=8),
                in_=xt_chunk_ap(0, cq))
        nc.sync.dma_start(out=bq_sb, in_=bq128[:, :])
        for cq in range(2, 4):
            nc.sync.dma_start(
                out=xt_sb[0][cq].rearrange("p (i s) -> p i s", i=8),
                in_=xt_chunk_ap(0, cq))
        wv_src = bass.AP(tensor=wv.tensor, offset=0,
                         ap=[[FPC, 128], [128 * FPC, 8], [1, FPC]])
        nc.sync.dma_start(
            out=wv_sb.rearrange("p (i f) -> p i f", i=8), in_=wv_src)

        def emit_late_loads():
            for cq in range(4):
                nc.sync.dma_start(
                    out=xt_sb[1][cq].rearrange("p (i s) -> p i s", i=8),
                    in_=xt_chunk_ap(1, cq))

        def emit_wout_loads():
            wout_src = bass.AP(tensor=wout.tensor, offset=0,
                               ap=[[D, 128], [128 * D, 8], [1, D]])
            nc.sync.dma_start(
                out=wout_sb.rearrange("p (i f) -> p i f", i=8), in_=wout_src)
            nc.sync.dma_start(out=bout_sb, in_=bout8[:, :])
            nc.sync.dma_start(out=ones32, in_=onesr[:, :])

        a2a_in = [dram.tile([8, HD, SC], FP8, tag=f"a2a_in{h}",
                            name=f"a2a_in{h}", bufs=1) for h in range(HPC)]
        a2a_out = [dram.tile([8, HD, SC], FP8, tag=f"a2a_out{h}",
                             name=f"a2a_out{h}", bufs=1) for h in range(HPC)]

        def emit_a2a(h):
            if with_collective:
                nc.gpsimd.collective_compute(
                    "AllToAll", mybir.AluOpType.bypass,
                    replica_groups=[list(range(8))],
                    ins=[a2a_in[h][:, :, :].opt()],
                    outs=[a2a_out[h][:, :, :].opt()])

        # rearranged views for DoubleRow pair slicing
        xt_r = [[xt_sb[b][cq].rearrange("p (i s) -> p i s", i=8)
                 for cq in range(4)] for b in range(B)]
        wqk_r = wqk_sb.rearrange("p (i f) -> p i f", i=8)
        wv_r = wv_sb.rearrange("p (i f) -> p i f", i=8)
        wout_r = wout_sb.rearrange("p (i f) -> p i f", i=8)
        q2r = [[q2s[b][h].rearrange("p (t s) -> p t s", t=2)
                for h in range(HPC)] for b in range(B)]
        k2r = [[k2s[b][h].rearrange("p (t s) -> p t s", t=2)
                for h in range(HPC)] for b in range(B)]
        v_r = [v_all[b].rearrange("p (kb h w) -> p kb h w", kb=NKB, h=HPC)
               for b in range(B)]

        cast_cnt = [0]

        def cast_eng():
            cast_cnt[0] += 1
            return nc.scalar if cast_cnt[0] % 2 else None


        # ------------- projections -------------
        def emit_qk(b, qn):
            # q and k into one [128,1024] psum -> single strided fp8 cast
            ps = pps.tile([128, 1024], F32, tag="scores", name="ps_qk")
            for m in range(2):
                for kp in range(4):
                    nc.tensor.matmul(
                        ps[:, m * 512:(m + 1) * 512],
                        wqk_r[:, 2 * kp:2 * kp + 2,
                              m * 128:(m + 1) * 128],
                        xt_r[b][qn][:, 2 * kp:2 * kp + 2, :],
                        start=(kp == 0), stop=(kp == 3), perf_mode=DRM)
            qk8 = qkstage[b].rearrange("p (m s) -> p m s", m=2)[
                :, :, qn * 512:(qn + 1) * 512]
            psr = ps.rearrange("p (m s) -> p m s", m=2)
            if cast_eng():
                nc.scalar.copy(qk8, psr)
            else:
                nc.vector.tensor_copy(qk8, psr)
            # bias row: (bq/4 . k) per head -> fp8 staging
            k8 = qkstage[b][:, S + qn * 512:S + (qn + 1) * 512]
            bps = pps.tile([2, 512], F32, tag="scores", name="bps")
            nc.tensor.matmul(bps, bq_sb, k8, start=True, stop=True)
            b8 = bstage[b][:, qn * 512:(qn + 1) * 512]
            if cast_eng():
                nc.scalar.copy(b8, bps)
            else:
                nc.vector.tensor_copy(b8, bps)

        def emit_qk_flush(b, half=None):
            # remap DMA per (m, h, t) + bias rows; half=0/1 flushes the
            # qn 0-1 / 2-3 column halves only (keys 0-1023 / 1024-2047)
            lo, hi = (0, S) if half is None else (half * 1024,
                                                 (half + 1) * 1024)
            for m, dst_l in ((0, q2s), (1, k2s)):
                for h in range(HPC):
                    for t in range(2):
                        nc.sync.dma_start(
                            out=dst_l[b][h][0:32, t * S + lo:t * S + hi],
                            in_=qkstage[b][h * 64 + t * 32:
                                           h * 64 + t * 32 + 32,
                                           m * S + lo:m * S + hi])
            for h in range(HPC):
                nc.sync.dma_start(out=k2s[b][h][32:33, lo:hi],
                                  in_=bstage[b][h:h + 1, lo:hi])

        def emit_v(b, cq):
            ps = pps.tile([128, 512], F32, tag="scores", name="ps_v")
            for kp in range(4):
                for t in range(4):
                    nc.tensor.matmul(
                        ps[:, t * 128:(t + 1) * 128],
                        xt_r[b][cq][:, 2 * kp:2 * kp + 2,
                                    t * 128:(t + 1) * 128],
                        wv_r[:, 2 * kp:2 * kp + 2, :],
                        start=(kp == 0), stop=(kp == 3), perf_mode=DRM)
            dst = v_r[b][:, cq * 4:(cq + 1) * 4, :, 0:HD]
            src = ps.rearrange("p (t h w) -> p t h w", t=4, h=HPC)
            if cast_eng():
                nc.scalar.copy(dst, src)
            else:
                nc.vector.tensor_copy(dst, src)

        # ------------- attention -------------
        def emit_exp(b, h, kb, ps_s, dst):
            if EXP_ENG[kb] == "A":
                nc.scalar.activation(dst, ps_s, Exp, scale=EXPSC)
            else:
                with nc.allow_low_precision(
                        reason="softmax exp via fp8 exp2 bit trick"):
                    nc.vector.tensor_scalar(
                        out=dst.bitcast(I8), in0=ps_s,
                        scalar1=A_TRICK, scalar2=B_TRICK,
                        op0=mybir.AluOpType.mult, op1=mybir.AluOpType.add)

        def emit_attn(b, h, qh, fillers=()):
            fillers = [e if isinstance(e, tuple) else (0, e)
                       for e in fillers]
            ps_o = pav.tile([VW, 1024], F32, tag="pso", name="ps_o")

            def emit_av(pair, exr):
                for q2 in range(2):
                    nc.tensor.matmul(
                        ps_o[:, q2 * 512:(q2 + 1) * 512],
                        v_r[b][:, 2 * pair:2 * pair + 2, h:h + 1, 0:VW],
                        exr[:, :, q2 * 512:(q2 + 1) * 512],
                        start=(pair == 0), stop=(pair == 7), perf_mode=DRM)

            prev = None
            for pair in range(8):
                while fillers and fillers[0][0] <= pair:
                    fillers.pop(0)[1]()
                ex = pexp.tile([128, 2048], FP8, tag="ex", name="ex")
                for j in range(2):
                    kb = 2 * pair + j
                    ps_s = pscore.tile([128, 1024], F32, tag="scores",
                                       name="ps_s")
                    for q2 in range(2):
                        nc.tensor.matmul(
                            ps_s[:, q2 * 512:(q2 + 1) * 512],
                            k2r[b][h][:, :, kb * 128:(kb + 1) * 128],
                            q2r[b][h][:, :, (qh * 2 + q2) * 512:
                                      (qh * 2 + q2 + 1) * 512],
                            start=True, stop=True, perf_mode=DRM)
                    emit_exp(b, h, kb, ps_s,
                             ex[:, j * 1024:(j + 1) * 1024])
                if prev is not None:
                    emit_av(*prev)
                prev = (pair, ex.rearrange("p (j q) -> p j q", j=2))
            emit_av(*prev)
            for _, f in fillers:
                f()
            # normalization: recip emitted eagerly (DVE, doesn't block PE);
            # the bc-broadcast + multiply are returned as a closure and run
            # as a filler inside the NEXT group so the PE queue never stalls
            # on the reciprocal.
            rec = pwork.tile([VW, 1024], mybir.dt.float32r, tag="rec",
                             name="rec")
            with nc.allow_low_precision(
                    reason="softmax denom recip rounded to f32r"):
                nc.vector.reciprocal(rec[HD:VW, :], ps_o[HD:VW, :])

            def finish_q2(q2):
                bc = pps.tile([HD, 512], F32, tag="scores", name="bc")
                nc.tensor.matmul(
                    bc, ones32[HD:VW, :],
                    rec[HD:VW, q2 * 512:(q2 + 1) * 512],
                    start=True, stop=True)
                nc.vector.tensor_mul(
                    att_sb[b][h][:, (qh * 2 + q2) * 512:
                                 (qh * 2 + q2 + 1) * 512],
                    ps_o[0:HD, q2 * 512:(q2 + 1) * 512], bc)
            return (lambda: finish_q2(0)), (lambda: finish_q2(1))

        def emit_ship(b, h, half=None):
            j0, j1 = (0, 4) if half is None else (2 * half, 2 * half + 2)
            nc.sync.dma_start(
                out=a2a_in[h][b * 4 + j0:b * 4 + j1, :, :].rearrange(
                    "j p s -> p j s"),
                in_=att_sb[b][h].rearrange(
                    "p (j s) -> p j s", j=4)[:, j0:j1, :])

        srcb = a2a_out if with_collective else a2a_in

        def emit_ao_load(h):
            src4 = srcb[h].rearrange("(j two) p s -> two p j s", two=2)
            nc.sync.dma_start(
                out=ao_sb[0:HD, 4 * h * SC:(4 * h + 4) * SC].rearrange(
                    "p (j s) -> p j s", j=4),
                in_=src4[0:1, :, :, :])
            nc.sync.dma_start(
                out=ao_sb[HD:128, 4 * h * SC:(4 * h + 4) * SC].rearrange(
                    "p (j s) -> p j s", j=4),
                in_=src4[1:2, :, :, :])

        ao_r = ao_sb.rearrange("p (i s) -> p i s", i=8)

        def emit_out1(g):
            sm, en = g // 2, g % 2
            ps = pps.tile([128, 512], F32, tag="scores", name="ps_out")
            for kp in range(2):
                nc.tensor.matmul(
                    ps,
                    ao_r[:, 2 * kp:2 * kp + 2, sm * 128:(sm + 1) * 128],
                    wout_r[:, 2 * kp:2 * kp + 2, en * 512:(en + 1) * 512],
                    start=(kp == 0), stop=(kp == 1), perf_mode=DRM)
            if cast_eng():
                nc.scalar.copy(part_sb[g], ps)
            else:
                nc.vector.tensor_copy(part_sb[g], ps)

        def emit_out2(g):
            sm, en = g // 2, g % 2
            ps = pps.tile([128, 512], F32, tag="scores", name="ps_out")
            for kp in range(2, 4):
                nc.tensor.matmul(
                    ps,
                    ao_r[:, 2 * kp:2 * kp + 2, sm * 128:(sm + 1) * 128],
                    wout_r[:, 2 * kp:2 * kp + 2, en * 512:(en + 1) * 512],
                    start=(kp == 2), stop=False, perf_mode=DRM)
            nc.tensor.matmul(
                ps, ones16,
                bout_sb[:, en * 512:(en + 1) * 512],
                start=False, stop=True)
            osb = pwork.tile([128, 512], F32, tag="osb", name="osb")
            nc.vector.tensor_add(osb, ps, part_sb[g])
            nc.sync.dma_start(
                out=out[sm * 128:(sm + 1) * 128, en * 512:(en + 1) * 512],
                in_=osb)

        def F(fn, *a):
            return lambda: fn(*a)

        # ------------- schedule -------------
        for qn in range(NQC):
            emit_qk(0, qn)
        emit_qk_flush(0)
        for cq in range(4):
            emit_v(0, cq)
        emit_late_loads()
        n00 = emit_attn(0, 0, 0)
        for qn in range(NQC):
            emit_qk(1, qn)
        emit_qk_flush(1)
        emit_wout_loads()
        n01 = emit_attn(0, 1, 0, fillers=[(3, n00[0]), (5, n00[1])])
        for cq in range(4):
            emit_v(1, cq)
        n02 = emit_attn(0, 0, 1, fillers=[(3, n01[0]), (5, n01[1])])
        n03 = emit_attn(0, 1, 1, fillers=[(3, n02[0]), (5, n02[1]),
                                          (3, F(emit_ship, 0, 0))])
        n10 = emit_attn(1, 0, 0, fillers=[(3, n03[0]), (5, n03[1]),
                                          (3, F(emit_ship, 0, 1))])
        n11 = emit_attn(1, 0, 1, fillers=[(3, n10[0]), (5, n10[1])])
        n12 = emit_attn(1, 1, 0, fillers=[(3, n11[0]), (5, n11[1]),
                                          (3, F(emit_ship, 1, 0)),
                                          (4, F(emit_a2a, 0)),
                                          (5, F(emit_ao_load, 0))])
        n13 = emit_attn(1, 1, 1, fillers=[(3, n12[0]), (5, n12[1]),
                                          (3, F(emit_out1, 0)),
                                          (3, F(emit_out1, 1)),
                                          (4, F(emit_out1, 2)),
                                          (4, F(emit_out1, 3)),
                                          (5, F(emit_out1, 4)),
                                          (5, F(emit_out1, 5)),
                                          (6, F(emit_ship, 1, 1, 0)),
                                          (6, F(emit_out1, 6)),
                                          (7, F(emit_out1, 7))])
        n13[0]()
        n13[1]()
        emit_ship(1, 1, 1)
        emit_a2a(1)
        emit_ao_load(1)
        for g in range(8):
            emit_out2(g)

    nc.compile()
    return nc


_NC_CACHE = {}


def _get_nc(with_collective: bool = True):
    key = bool(with_collective)
    if key not in _NC_CACHE:
        _NC_CACHE[key] = _build_nc(with_collective)
    return _NC_CACHE[key]


def make_in_maps(x, w_qkv, b_qkv, w_out, b_out):
    """Host-side sharding/prep. Returns per-core input dicts."""
    x = np.asarray(x, dtype=np.float32)
    w_qkv = np.asarray(w_qkv, dtype=np.float32)
    b_qkv = np.asarray(b_qkv, dtype=np.float32)
    w_out = np.asarray(w_out, dtype=np.float32)
    b_out = np.asarray(b_out, dtype=np.float32)

    wq = w_qkv[0:D].reshape(H, HD, D)
    wk = w_qkv[D:2 * D].reshape(H, HD, D)
    wv_ = w_qkv[2 * D:3 * D].reshape(H, HD, D)
    bq = b_qkv[0:D].reshape(H, HD)
    bv = b_qkv[2 * D:3 * D]
    b_eff = b_out + w_out @ bv

    perm = np.concatenate(
        [np.arange(h * HD, (h + 1) * HD) for h in range(0, H, 2)]
        + [np.arange(h * HD, (h + 1) * HD) for h in range(1, H, 2)])
    wout_t = np.ascontiguousarray(w_out.T[perm]).astype(NPFP8)
    bout_t = np.tile((b_eff * 2.0).astype(NPFP8)[None, :], (8, 1))

    xt_all = np.ascontiguousarray(
        np.concatenate([x[0].T, x[1].T], axis=1)).astype(NPFP8)

    in_maps = []
    for c in range(NCORES):
        hs = slice(c * HPC, (c + 1) * HPC)
        wq_c = (wq[hs].reshape(FPC, D) * 32.0).T
        wk_c = (wk[hs].reshape(FPC, D) * 32.0).T
        wqk_c = np.concatenate([wq_c, wk_c], axis=1).astype(NPFP8)
        # bq128: col h = bq*8 on rows h*64..h*64+64, else 0
        bq_c = np.zeros((128, 2), dtype=np.float32)
        for h in range(HPC):
            bq_c[h * 64:(h + 1) * 64, h] = bq[c * HPC + h] * 8.0
        wv_c = (wv_[hs].reshape(FPC, D) * 32.0).T.astype(NPFP8)
        in_maps.append({
            "onesr": np.full((VW, HD), 1.0 / 32.0, dtype=np.float32),
            "xt": xt_all,
            "wqk": np.ascontiguousarray(wqk_c),
            "bq128": bq_c.astype(NPFP8),
            "wv": np.ascontiguousarray(wv_c),
            "wout": wout_t,
            "bout8": bout_t,
        })
    return in_maps


def assemble_output(results):
    out = np.empty((B, S, D), dtype=np.float32)
    for c in range(NCORES):
        b, sg = c // 4, c % 4
        out[b, sg * SC:(sg + 1) * SC, :] = results[c]["out"]
    return out


def kernel(x, mask, w_qkv, b_qkv, w_out, b_out):
    nc = _get_nc(True)
    in_maps = make_in_maps(x, w_qkv, b_qkv, w_out, b_out)
    res = run_bass_kernel_spmd(nc, in_maps, core_ids=list(range(NCORES)))
    return assemble_output(res.results)


# revision 15
# speedup vs baseline: 1.0248x; 1.0248x over previous
"""Multi-head attention on 8 Trainium2 NeuronCores (Bass/Tile), fp8 edition.

Problem: x[2,2048,1024] -> qkv proj (16 heads, hd=64) -> softmax(QK^T/8)V
-> out proj.  mask is all-ones (per spec) and is ignored.

Sharding: core c owns heads {2c, 2c+1} for BOTH batches (tensor-parallel
QKV + attention).  An 8-core AllToAll converts the head-sharded attention
output into a sequence-sharded full-feature activation; core c ends up
with global row chunk c (batch c//4, rows (c%4)*512..) and computes the
output projection full-width.

All matmuls run in fp8e4m3 with DoubleRow perf mode (2 K-subtiles per
pass).  Scales: weights x32 into fp8 range; scores accumulate f32 as
8192*score_true; exp applies 1/8192.  Biases:
  - k bias dropped (softmax-invariant along keys),
  - q bias folded into scores via an augmented 33rd contraction row
    (k'' row 32 holds fp8(bq.k)/4, q'' row 32 holds 4.0),
  - v bias folded into the output-projection bias (b_eff = b_out + W_o@b_v),
  - b_eff enters the out-proj PSUM via an fp8 ones-chunk matmul.
exp is computed per key-block [128,1024] and split between the Act engine
(table Exp, fp8 out) and the DVE (bit-exact int8 exp2 trick: fp8 bit
pattern = round(score*8*log2e/8192 + 55.656), verified vs HW).

PSUM: scores [128,1024] x2 (4 banks) + AV [65,1024] (2) + work [*,512]
x2 (2) = 8 banks exactly.
"""

import numpy as np
from contextlib import ExitStack

import concourse.bass as bass
import concourse.mybir as mybir
import concourse.tile as tile
from concourse import bacc
from concourse.bass_utils import run_bass_kernel_spmd

BF16 = mybir.dt.bfloat16
F32 = mybir.dt.float32
FP8 = mybir.dt.float8e4
I8 = mybir.dt.int8
NPFP8 = mybir.dt.np(FP8)
DRM = mybir.MatmulPerfMode.DoubleRow
Exp = mybir.ActivationFunctionType.Exp

D, H, HD, B, S = 1024, 16, 64, 2, 2048
NCORES = 8
HPC = 2              # heads per core
FPC = HPC * HD       # 128 features per core
SS = B * S           # 4096 stacked sequence (batch-major)
SC = 512             # output rows per core (post all-to-all)
NKB = S // 128       # 16 key blocks per batch
NQC = S // 512       # 4 query chunks per batch
VW = HD + 1          # live v columns per head (vd + ones)
VWP = 80             # padded v block stride (16-aligned)

EXPSC = 1.0 / 8192.0
A_TRICK = 8.0 * np.log2(np.e) / 8192.0
B_TRICK = 56.0 - 0.344   # fp8 exponent offset + PWL centering

# exp engine per kb: 'A' = Act (table exp), 'D' = DVE (int8 trick)
EXP_ENG = "ADAADADAADADAADA"


def _build_nc(with_collective: bool = True):
    nc = bacc.Bacc("TRN2", target_bir_lowering=False, debug=False,
                   num_devices=NCORES)
    xt = nc.dram_tensor("xt", [D, SS], FP8, kind="ExternalInput").ap()
    wqk = nc.dram_tensor("wqk", [D, 2 * FPC], FP8, kind="ExternalInput").ap()
    bq128 = nc.dram_tensor("bq128", [128, 2], FP8, kind="ExternalInput").ap()
    wv = nc.dram_tensor("wv", [D, FPC], FP8, kind="ExternalInput").ap()
    wout = nc.dram_tensor("wout", [D, D], FP8, kind="ExternalInput").ap()
    bout8 = nc.dram_tensor("bout8", [8, D], FP8, kind="ExternalInput").ap()
    onesr = nc.dram_tensor("onesr", [VW, HD], mybir.dt.float32r,
                           kind="ExternalInput").ap()
    out = nc.dram_tensor("out", [SC, D], F32, kind="ExternalOutput").ap()

    with ExitStack() as ctx:
        tc = ctx.enter_context(tile.TileContext(nc))
        persist = ctx.enter_context(tc.tile_pool(name="persist", bufs=1))
        pexp = ctx.enter_context(tc.tile_pool(name="pexp", bufs=6))
        pwork = ctx.enter_context(tc.tile_pool(name="pwork", bufs=3))
        pscore = ctx.enter_context(tc.tile_pool(name="pscore", bufs=3,
                                                space="PSUM"))
        pav = ctx.enter_context(tc.tile_pool(name="pav", bufs=1,
                                             space="PSUM"))
        pps = pscore
        dram = ctx.enter_context(tc.tile_pool(name="dram", bufs=2,
                                              space="DRAM"))

        # ---------------- persistent SBUF ----------------
        xt_sb = [[persist.tile([128, 8 * 512], FP8, tag=f"xt{b}_{cq}",
                               name=f"xt{b}_{cq}") for cq in range(4)]
                 for b in range(B)]
        wqk_sb = persist.tile([128, 8 * 2 * FPC], FP8, tag="wqk",
                              name="wqk_sb")
        bq_sb = persist.tile([128, 2], FP8, tag="bq", name="bq_sb")
        wv_sb = persist.tile([128, 8 * FPC], FP8, tag="wv", name="wv_sb")
        wout_sb = persist.tile([128, 8 * D], FP8, tag="wout", name="wout_sb")
        bout_sb = persist.tile([8, D], FP8, tag="bout", name="bout_sb")
        ones16 = persist.tile([8, 128], FP8, tag="ones16", name="ones16")
        nc.gpsimd.memset(ones16, 0.0625)

        # q2s/k2s[b][h]: [33, 2*2048] — hd-half t at cols t*2048;
        # row 32: q'' ones (4.0 at t0) / k'' bias row (device-filled)
        q2s = [[persist.tile([33, 2 * S], FP8, tag=f"q2_{b}_{h}",
                             name=f"q2_{b}_{h}") for h in range(HPC)]
               for b in range(B)]
        k2s = [[persist.tile([33, 2 * S], FP8, tag=f"k2_{b}_{h}",
                             name=f"k2_{b}_{h}") for h in range(HPC)]
               for b in range(B)]
        for b in range(B):
            for h in range(HPC):
                nc.gpsimd.memset(q2s[b][h][32:33, 0:S], 4.0)
                nc.gpsimd.memset(q2s[b][h][32:33, S:2 * S], 0.0)
                nc.gpsimd.memset(k2s[b][h][32:33, S:2 * S], 0.0)

        # v_all[b]: [128, kb(16) x h(2) x 80]; col 64 = ones
        v_all = [persist.tile([128, NKB * HPC * VWP], FP8, tag=f"v{b}",
                              name=f"v{b}") for b in range(B)]
        for b in range(B):
            vr = v_all[b].rearrange("p (kb h w) -> p kb h w", kb=NKB, h=HPC)
            nc.gpsimd.memset(vr[:, :, :, HD:VW], 1.0)

        att_sb = [[persist.tile([64, S], FP8, tag=f"att{b}_{h}",
                                name=f"att{b}_{h}") for h in range(HPC)]
                  for b in range(B)]
        ones32 = persist.tile([VW, HD], mybir.dt.float32r, tag="ones32",
                              name="ones32")
        ao_sb = persist.tile([128, 8 * SC], FP8, tag="ao", name="ao_sb")
        part_sb = [persist.tile([128, 512], BF16, tag=f"part{g}",
                                name=f"part{g}") for g in range(8)]
        qkstage = [persist.tile([128, 2 * S], FP8, tag=f"qkst{b}",
                                name=f"qkst{b}") for b in range(B)]
        bstage = [persist.tile([2, S], FP8, tag=f"bst{b}", name=f"bst{b}")
                  for b in range(B)]

        # ---------------- loads (ordered by first use) ----------------
        def xt_chunk_ap(b, cq):
            return bass.AP(tensor=xt.tensor, offset=b * S + cq * 512,
                           ap=[[SS, 128], [128 * SS, 8], [1, 512]])

        wqk_src = bass.AP(tensor=wqk.tensor, offset=0,
                          ap=[[2 * FPC, 128], [128 * 2 * FPC, 8], [1, 2 * FPC]])
        nc.sync.dma_start(
            out=wqk_sb.rearrange("p (i f) -> p i f", i=8), in_=wqk_src)
        for cq in range(2):
            nc.sync.dma_start(
                out=xt_sb[0][cq].rearrange("p (i s) -> p i s", i=8),
                in_=xt_chunk_ap(0, cq))
        nc.sync.dma_start(out=bq_sb, in_=bq128[:, :])
        for cq in range(2, 4):
            nc.sync.dma_start(
                out=xt_sb[0][cq].rearrange("p (i s) -> p i s", i=8),
                in_=xt_chunk_ap(0, cq))
        wv_src = bass.AP(tensor=wv.tensor, offset=0,
                         ap=[[FPC, 128], [128 * FPC, 8], [1, FPC]])
        nc.sync.dma_start(
            out=wv_sb.rearrange("p (i f) -> p i f", i=8), in_=wv_src)

        def emit_late_loads():
            for cq in range(4):
                nc.sync.dma_start(
                    out=xt_sb[1][cq].rearrange("p (i s) -> p i s", i=8),
                    in_=xt_chunk_ap(1, cq))

        def emit_wout_loads():
            wout_src = bass.AP(tensor=wout.tensor, offset=0,
                               ap=[[D, 128], [128 * D, 8], [1, D]])
            nc.sync.dma_start(
                out=wout_sb.rearrange("p (i f) -> p i f", i=8), in_=wout_src)
            nc.sync.dma_start(out=bout_sb, in_=bout8[:, :])
            nc.sync.dma_start(out=ones32, in_=onesr[:, :])

        a2a_in = [dram.tile([8, HD, SC], FP8, tag=f"a2a_in{h}",
                            name=f"a2a_in{h}", bufs=1) for h in range(HPC)]
        a2a_out = [dram.tile([8, HD, SC], FP8, tag=f"a2a_out{h}",
                             name=f"a2a_out{h}", bufs=1) for h in range(HPC)]

        def emit_a2a(h):
            if with_collective:
                nc.gpsimd.collective_compute(
                    "AllToAll", mybir.AluOpType.bypass,
                    replica_groups=[list(range(8))],
                    ins=[a2a_in[h][:, :, :].opt()],
                    outs=[a2a_out[h][:, :, :].opt()])

        # rearranged views for DoubleRow pair slicing
        xt_r = [[xt_sb[b][cq].rearrange("p (i s) -> p i s", i=8)
                 for cq in range(4)] for b in range(B)]
        wqk_r = wqk_sb.rearrange("p (i f) -> p i f", i=8)
        wv_r = wv_sb.rearrange("p (i f) -> p i f", i=8)
        wout_r = wout_sb.rearrange("p (i f) -> p i f", i=8)
        q2r = [[q2s[b][h].rearrange("p (t s) -> p t s", t=2)
                for h in range(HPC)] for b in range(B)]
        k2r = [[k2s[b][h].rearrange("p (t s) -> p t s", t=2)
                for h in range(HPC)] for b in range(B)]
        v_r = [v_all[b].rearrange("p (kb h w) -> p kb h w", kb=NKB, h=HPC)
               for b in range(B)]

        cast_cnt = [0]

        def cast_eng():
            cast_cnt[0] += 1
            return nc.scalar if cast_cnt[0] % 2 else None


        # ------------- projections -------------
        def emit_qk(b, qn):
            # q and k into one [128,1024] psum -> single strided fp8 cast
            ps = pps.tile([128, 1024], F32, tag="scores", name="ps_qk")
            for m in range(2):
                for kp in range(4):
                    nc.tensor.matmul(
                        ps[:, m * 512:(m + 1) * 512],
                        wqk_r[:, 2 * kp:2 * kp + 2,
                              m * 128:(m + 1) * 128],
                        xt_r[b][qn][:, 2 * kp:2 * kp + 2, :],
                        start=(kp == 0), stop=(kp == 3), perf_mode=DRM)
            qk8 = qkstage[b].rearrange("p (m s) -> p m s", m=2)[
                :, :, qn * 512:(qn + 1) * 512]
            psr = ps.rearrange("p (m s) -> p m s", m=2)
            if cast_eng():
                nc.scalar.copy(qk8, psr)
            else:
                nc.vector.tensor_copy(qk8, psr)
            # bias row: (bq/4 . k) per head -> fp8 staging
            k8 = qkstage[b][:, S + qn * 512:S + (qn + 1) * 512]
            bps = pps.tile([2, 512], F32, tag="scores", name="bps")
            nc.tensor.matmul(bps, bq_sb, k8, start=True, stop=True)
            b8 = bstage[b][:, qn * 512:(qn + 1) * 512]
            if cast_eng():
                nc.scalar.copy(b8, bps)
            else:
                nc.vector.tensor_copy(b8, bps)

        def emit_qk_flush(b, half=None):
            # remap DMA per (m, h, t) + bias rows; half=0/1 flushes the
            # qn 0-1 / 2-3 column halves only (keys 0-1023 / 1024-2047)
            lo, hi = (0, S) if half is None else (half * 1024,
                                                 (half + 1) * 1024)
            for m, dst_l in ((0, q2s), (1, k2s)):
                for h in range(HPC):
                    for t in range(2):
                        nc.sync.dma_start(
                            out=dst_l[b][h][0:32, t * S + lo:t * S + hi],
                            in_=qkstage[b][h * 64 + t * 32:
                                           h * 64 + t * 32 + 32,
                                           m * S + lo:m * S + hi])
            for h in range(HPC):
                nc.sync.dma_start(out=k2s[b][h][32:33, lo:hi],
                                  in_=bstage[b][h:h + 1, lo:hi])

        def emit_v(b, cq):
            ps = pps.tile([128, 512], F32, tag="scores", name="ps_v")
            for kp in range(4):
                for t in range(4):
                    nc.tensor.matmul(
                        ps[:, t * 128:(t + 1) * 128],
                        xt_r[b][cq][:, 2 * kp:2 * kp + 2,
                                    t * 128:(t + 1) * 128],
                        wv_r[:, 2 * kp:2 * kp + 2, :],
                        start=(kp == 0), stop=(kp == 3), perf_mode=DRM)
            dst = v_r[b][:, cq * 4:(cq + 1) * 4, :, 0:HD]
            src = ps.rearrange("p (t h w) -> p t h w", t=4, h=HPC)
            if cast_eng():
                nc.scalar.copy(dst, src)
            else:
                nc.vector.tensor_copy(dst, src)

        # ------------- attention -------------
        def emit_exp(b, h, kb, ps_s, dst):
            if EXP_ENG[kb] == "A":
                nc.scalar.activation(dst, ps_s, Exp, scale=EXPSC)
            else:
                with nc.allow_low_precision(
                        reason="softmax exp via fp8 exp2 bit trick"):
                    nc.vector.tensor_scalar(
                        out=dst.bitcast(I8), in0=ps_s,
                        scalar1=A_TRICK, scalar2=B_TRICK,
                        op0=mybir.AluOpType.mult, op1=mybir.AluOpType.add)

        def emit_attn(b, h, qh, fillers=()):
            fillers = [e if isinstance(e, tuple) else (0, e)
                       for e in fillers]
            ps_o = pav.tile([VW, 1024], F32, tag="pso", name="ps_o")

            def emit_av(pair, exr):
                for q2 in range(2):
                    nc.tensor.matmul(
                        ps_o[:, q2 * 512:(q2 + 1) * 512],
                        v_r[b][:, 2 * pair:2 * pair + 2, h:h + 1, 0:VW],
                        exr[:, :, q2 * 512:(q2 + 1) * 512],
                        start=(pair == 0), stop=(pair == 7), perf_mode=DRM)

            prev = None
            for pair in range(8):
                while fillers and fillers[0][0] <= pair:
                    fillers.pop(0)[1]()
                ex = pexp.tile([128, 2048], FP8, tag="ex", name="ex")
                for j in range(2):
                    kb = 2 * pair + j
                    ps_s = pscore.tile([128, 1024], F32, tag="scores",
                                       name="ps_s")
                    for q2 in range(2):
                        nc.tensor.matmul(
                            ps_s[:, q2 * 512:(q2 + 1) * 512],
                            k2r[b][h][:, :, kb * 128:(kb + 1) * 128],
                            q2r[b][h][:, :, (qh * 2 + q2) * 512:
                                      (qh * 2 + q2 + 1) * 512],
                            start=True, stop=True, perf_mode=DRM)
                    emit_exp(b, h, kb, ps_s,
                             ex[:, j * 1024:(j + 1) * 1024])
                if prev is not None:
                    emit_av(*prev)
                prev = (pair, ex.rearrange("p (j q) -> p j q", j=2))
            emit_av(*prev)
            for _, f in fillers:
                f()
            # normalization: recip emitted eagerly (DVE, doesn't block PE);
            # the bc-broadcast + multiply are returned as a closure and run
            # as a filler inside the NEXT group so the PE queue never stalls
            # on the reciprocal.
            rec = pwork.tile([VW, 1024], mybir.dt.float32r, tag="rec",
                             name="rec")
            with nc.allow_low_precision(
                    reason="softmax denom recip rounded to f32r"):
                nc.vector.reciprocal(rec[HD:VW, :], ps_o[HD:VW, :])

            def finish_q2(q2):
                bc = pps.tile([HD, 512], F32, tag="scores", name="bc")
                nc.tensor.matmul(
                    bc, ones32[HD:VW, :],
                    rec[HD:VW, q2 * 512:(q2 + 1) * 512],
                    start=True, stop=True)
                nc.vector.tensor_mul(
                    att_sb[b][h][:, (qh * 2 + q2) * 512:
                                 (qh * 2 + q2 + 1) * 512],
                    ps_o[0:HD, q2 * 512:(q2 + 1) * 512], bc)
            return (lambda: finish_q2(0)), (lambda: finish_q2(1))

        def emit_ship(b, h, half=None):
            j0, j1 = (0, 4) if half is None else (2 * half, 2 * half + 2)
            nc.sync.dma_start(
                out=a2a_in[h][b * 4 + j0:b * 4 + j1, :, :].rearrange(
                    "j p s -> p j s"),
                in_=att_sb[b][h].rearrange(
                    "p (j s) -> p j s", j=4)[:, j0:j1, :])

        srcb = a2a_out if with_collective else a2a_in

        def emit_ao_load(h):
            src4 = srcb[h].rearrange("(j two) p s -> two p j s", two=2)
            nc.sync.dma_start(
                out=ao_sb[0:HD, 4 * h * SC:(4 * h + 4) * SC].rearrange(
                    "p (j s) -> p j s", j=4),
                in_=src4[0:1, :, :, :])
            nc.sync.dma_start(
                out=ao_sb[HD:128, 4 * h * SC:(4 * h + 4) * SC].rearrange(
                    "p (j s) -> p j s", j=4),
                in_=src4[1:2, :, :, :])

        ao_r = ao_sb.rearrange("p (i s) -> p i s", i=8)

        def emit_out1(g):
            sm, en = g // 2, g % 2
            ps = pps.tile([128, 512], F32, tag="scores", name="ps_out")
            for kp in range(2):
                nc.tensor.matmul(
                    ps,
                    ao_r[:, 2 * kp:2 * kp + 2, sm * 128:(sm + 1) * 128],
                    wout_r[:, 2 * kp:2 * kp + 2, en * 512:(en + 1) * 512],
                    start=(kp == 0), stop=(kp == 1), perf_mode=DRM)
            if cast_eng():
                nc.scalar.copy(part_sb[g], ps)
            else:
                nc.vector.tensor_copy(part_sb[g], ps)

        def emit_out2(g):
            sm, en = g // 2, g % 2
            ps = pps.tile([128, 512], F32, tag="scores", name="ps_out")
            for kp in range(2, 4):
                nc.tensor.matmul(
                    ps,
                    ao_r[:, 2 * kp:2 * kp + 2, sm * 128:(sm + 1) * 128],
                    wout_r[:, 2 * kp:2 * kp + 2, en * 512:(en + 1) * 512],
                    start=(kp == 2), stop=False, perf_mode=DRM)
            nc.tensor.matmul(
                ps, ones16,
                bout_sb[:, en * 512:(en + 1) * 512],
                start=False, stop=True)
            osb = pwork.tile([128, 512], F32, tag="osb", name="osb")
            nc.vector.tensor_add(osb, ps, part_sb[g])
            nc.sync.dma_start(
                out=out[sm * 128:(sm + 1) * 128, en * 512:(en + 1) * 512],
                in_=osb)

        def F(fn, *a):
            return lambda: fn(*a)

        # ------------- schedule -------------
        for qn in range(NQC):
            emit_qk(0, qn)
        emit_qk_flush(0)
        for cq in range(4):
            emit_v(0, cq)
        emit_late_loads()
        n00 = emit_attn(0, 0, 0)
        for qn in range(NQC):
            emit_qk(1, qn)
        emit_qk_flush(1)
        emit_wout_loads()
        n01 = emit_attn(0, 1, 0, fillers=[(2, n00[0]), (4, n00[1])])
        for cq in range(4):
            emit_v(1, cq)
        n02 = emit_attn(0, 0, 1, fillers=[(2, n01[0]), (4, n01[1])])
        n03 = emit_attn(0, 1, 1, fillers=[(2, n02[0]), (4, n02[1]),
                                          (3, F(emit_ship, 0, 0))])
        n10 = emit_attn(1, 0, 0, fillers=[(2, n03[0]), (4, n03[1]),
                                          (3, F(emit_ship, 0, 1))])
        n11 = emit_attn(1, 0, 1, fillers=[(2, n10[0]), (4, n10[1])])
        n12 = emit_attn(1, 1, 0, fillers=[(2, n11[0]), (4, n11[1]),
                                          (3, F(emit_ship, 1, 0)),
                                          (4, F(emit_a2a, 0)),
                                          (5, F(emit_ao_load, 0))])
        n13 = emit_attn(1, 1, 1, fillers=[(2, n12[0]), (4, n12[1]),
                                          (3, F(emit_out1, 0)),
                                          (3, F(emit_out1, 1)),
                                          (4, F(emit_out1, 2)),
                                          (4, F(emit_out1, 3)),
                                          (5, F(emit_out1, 4)),
                                          (5, F(emit_out1, 5)),
                                          (6, F(emit_ship, 1, 1, 0)),
                                          (6, F(emit_out1, 6)),
                                          (7, F(emit_out1, 7))])
        n13[0]()
        n13[1]()
        emit_ship(1, 1, 1)
        emit_a2a(1)
        emit_ao_load(1)
        for g in range(8):
            emit_out2(g)

    nc.compile()
    return nc


_NC_CACHE = {}


def _get_nc(with_collective: bool = True):
    key = bool(with_collective)
    if key not in _NC_CACHE:
        _NC_CACHE[key] = _build_nc(with_collective)
    return _NC_CACHE[key]


def make_in_maps(x, w_qkv, b_qkv, w_out, b_out):
    """Host-side sharding/prep. Returns per-core input dicts."""
    x = np.asarray(x, dtype=np.float32)
    w_qkv = np.asarray(w_qkv, dtype=np.float32)
    b_qkv = np.asarray(b_qkv, dtype=np.float32)
    w_out = np.asarray(w_out, dtype=np.float32)
    b_out = np.asarray(b_out, dtype=np.float32)

    wq = w_qkv[0:D].reshape(H, HD, D)
    wk = w_qkv[D:2 * D].reshape(H, HD, D)
    wv_ = w_qkv[2 * D:3 * D].reshape(H, HD, D)
    bq = b_qkv[0:D].reshape(H, HD)
    bv = b_qkv[2 * D:3 * D]
    b_eff = b_out + w_out @ bv

    perm = np.concatenate(
        [np.arange(h * HD, (h + 1) * HD) for h in range(0, H, 2)]
        + [np.arange(h * HD, (h + 1) * HD) for h in range(1, H, 2)])
    wout_t = np.ascontiguousarray(w_out.T[perm]).astype(NPFP8)
    bout_t = np.tile((b_eff * 2.0).astype(NPFP8)[None, :], (8, 1))

    xt_all = np.ascontiguousarray(
        np.concatenate([x[0].T, x[1].T], axis=1)).astype(NPFP8)

    in_maps = []
    for c in range(NCORES):
        hs = slice(c * HPC, (c + 1) * HPC)
        wq_c = (wq[hs].reshape(FPC, D) * 32.0).T
        wk_c = (wk[hs].reshape(FPC, D) * 32.0).T
        wqk_c = np.concatenate([wq_c, wk_c], axis=1).astype(NPFP8)
        # bq128: col h = bq*8 on rows h*64..h*64+64, else 0
        bq_c = np.zeros((128, 2), dtype=np.float32)
        for h in range(HPC):
            bq_c[h * 64:(h + 1) * 64, h] = bq[c * HPC + h] * 8.0
        wv_c = (wv_[hs].reshape(FPC, D) * 32.0).T.astype(NPFP8)
        in_maps.append({
            "onesr": np.full((VW, HD), 1.0 / 32.0, dtype=np.float32),
            "xt": xt_all,
            "wqk": np.ascontiguousarray(wqk_c),
            "bq128": bq_c.astype(NPFP8),
            "wv": np.ascontiguousarray(wv_c),
            "wout": wout_t,
            "bout8": bout_t,
        })
    return in_maps


def assemble_output(results):
    out = np.empty((B, S, D), dtype=np.float32)
    for c in range(NCORES):
        b, sg = c // 4, c % 4
        out[b, sg * SC:(sg + 1) * SC, :] = results[c]["out"]
    return out


def kernel(x, mask, w_qkv, b_qkv, w_out, b_out):
    nc = _get_nc(True)
    in_maps = make_in_maps(x, w_qkv, b_qkv, w_out, b_out)
    res = run_bass_kernel_spmd(nc, in_maps, core_ids=list(range(NCORES)))
    return assemble_output(res.results)


# revision 16
# speedup vs baseline: 1.0338x; 1.0088x over previous
"""Multi-head attention on 8 Trainium2 NeuronCores (Bass/Tile), fp8 edition.

Problem: x[2,2048,1024] -> qkv proj (16 heads, hd=64) -> softmax(QK^T/8)V
-> out proj.  mask is all-ones (per spec) and is ignored.

Sharding: core c owns heads {2c, 2c+1} for BOTH batches (tensor-parallel
QKV + attention).  An 8-core AllToAll converts the head-sharded attention
output into a sequence-sharded full-feature activation; core c ends up
with global row chunk c (batch c//4, rows (c%4)*512..) and computes the
output projection full-width.

All matmuls run in fp8e4m3 with DoubleRow perf mode (2 K-subtiles per
pass).  Scales: weights x32 into fp8 range; scores accumulate f32 as
8192*score_true; exp applies 1/8192.  Biases:
  - k bias dropped (softmax-invariant along keys),
  - q bias folded into scores via an augmented 33rd contraction row
    (k'' row 32 holds fp8(bq.k)/4, q'' row 32 holds 4.0),
  - v bias folded into the output-projection bias (b_eff = b_out + W_o@b_v),
  - b_eff enters the out-proj PSUM via an fp8 ones-chunk matmul.
exp is computed per key-block [128,1024] and split between the Act engine
(table Exp, fp8 out) and the DVE (bit-exact int8 exp2 trick: fp8 bit
pattern = round(score*8*log2e/8192 + 55.656), verified vs HW).

PSUM: scores [128,1024] x2 (4 banks) + AV [65,1024] (2) + work [*,512]
x2 (2) = 8 banks exactly.
"""

import numpy as np
from contextlib import ExitStack

import concourse.bass as bass
import concourse.mybir as mybir
import concourse.tile as tile
from concourse import bacc
from concourse.bass_utils import run_bass_kernel_spmd

BF16 = mybir.dt.bfloat16
F32 = mybir.dt.float32
FP8 = mybir.dt.float8e4
I8 = mybir.dt.int8
NPFP8 = mybir.dt.np(FP8)
DRM = mybir.MatmulPerfMode.DoubleRow
Exp = mybir.ActivationFunctionType.Exp

D, H, HD, B, S = 1024, 16, 64, 2, 2048
NCORES = 8
HPC = 2              # heads per core
FPC = HPC * HD       # 128 features per core
SS = B * S           # 4096 stacked sequence (batch-major)
SC = 512             # output rows per core (post all-to-all)
NKB = S // 128       # 16 key blocks per batch
NQC = S // 512       # 4 query chunks per batch
VW = HD + 1          # live v columns per head (vd + ones)
VWP = 80             # padded v block stride (16-aligned)

EXPSC = 1.0 / 8192.0
A_TRICK = 8.0 * np.log2(np.e) / 8192.0
B_TRICK = 56.0 - 0.344   # fp8 exponent offset + PWL centering

# exp engine per kb: 'A' = Act (table exp), 'D' = DVE (int8 trick)
EXP_ENG = "ADAADADAADADAADA"


def _build_nc(with_collective: bool = True):
    nc = bacc.Bacc("TRN2", target_bir_lowering=False, debug=False,
                   num_devices=NCORES)
    xt = nc.dram_tensor("xt", [D, SS], FP8, kind="ExternalInput").ap()
    wqk = nc.dram_tensor("wqk", [D, 2 * FPC], FP8, kind="ExternalInput").ap()
    bq128 = nc.dram_tensor("bq128", [128, 2], FP8, kind="ExternalInput").ap()
    wv = nc.dram_tensor("wv", [D, FPC], FP8, kind="ExternalInput").ap()
    wout = nc.dram_tensor("wout", [D, D], FP8, kind="ExternalInput").ap()
    bout8 = nc.dram_tensor("bout8", [8, D], FP8, kind="ExternalInput").ap()
    onesr = nc.dram_tensor("onesr", [VW, HD], mybir.dt.float32r,
                           kind="ExternalInput").ap()
    out = nc.dram_tensor("out", [SC, D], F32, kind="ExternalOutput").ap()

    with ExitStack() as ctx:
        tc = ctx.enter_context(tile.TileContext(nc))
        persist = ctx.enter_context(tc.tile_pool(name="persist", bufs=1))
        pexp = ctx.enter_context(tc.tile_pool(name="pexp", bufs=6))
        pwork = ctx.enter_context(tc.tile_pool(name="pwork", bufs=3))
        pscore = ctx.enter_context(tc.tile_pool(name="pscore", bufs=3,
                                                space="PSUM"))
        pav = ctx.enter_context(tc.tile_pool(name="pav", bufs=1,
                                             space="PSUM"))
        pps = pscore
        dram = ctx.enter_context(tc.tile_pool(name="dram", bufs=2,
                                              space="DRAM"))

        # ---------------- persistent SBUF ----------------
        xt_sb = [[persist.tile([128, 8 * 512], FP8, tag=f"xt{b}_{cq}",
                               name=f"xt{b}_{cq}") for cq in range(4)]
                 for b in range(B)]
        wqk_sb = persist.tile([128, 8 * 2 * FPC], FP8, tag="wqk",
                              name="wqk_sb")
        bq_sb = persist.tile([128, 2], FP8, tag="bq", name="bq_sb")
        wv_sb = persist.tile([128, 8 * FPC], FP8, tag="wv", name="wv_sb")
        wout_sb = persist.tile([128, 8 * D], FP8, tag="wout", name="wout_sb")
        bout_sb = persist.tile([8, D], FP8, tag="bout", name="bout_sb")
        ones16 = persist.tile([8, 128], FP8, tag="ones16", name="ones16")
        nc.gpsimd.memset(ones16, 0.0625)

        # q2s/k2s[b][h]: [33, 2*2048] — hd-half t at cols t*2048;
        # row 32: q'' ones (4.0 at t0) / k'' bias row (device-filled)
        q2s = [[persist.tile([33, 2 * S], FP8, tag=f"q2_{b}_{h}",
                             name=f"q2_{b}_{h}") for h in range(HPC)]
               for b in range(B)]
        k2s = [[persist.tile([33, 2 * S], FP8, tag=f"k2_{b}_{h}",
                             name=f"k2_{b}_{h}") for h in range(HPC)]
               for b in range(B)]
        for b in range(B):
            for h in range(HPC):
                nc.gpsimd.memset(q2s[b][h][32:33, 0:S], 4.0)
                nc.gpsimd.memset(q2s[b][h][32:33, S:2 * S], 0.0)
                nc.gpsimd.memset(k2s[b][h][32:33, S:2 * S], 0.0)

        # v_all[b]: [128, kb(16) x h(2) x 80]; col 64 = ones
        v_all = [persist.tile([128, NKB * HPC * VWP], FP8, tag=f"v{b}",
                              name=f"v{b}") for b in range(B)]
        for b in range(B):
            vr = v_all[b].rearrange("p (kb h w) -> p kb h w", kb=NKB, h=HPC)
            nc.gpsimd.memset(vr[:, :, :, HD:VW], 1.0)

        att_sb = [[persist.tile([64, S], FP8, tag=f"att{b}_{h}",
                                name=f"att{b}_{h}") for h in range(HPC)]
                  for b in range(B)]
        ones32 = persist.tile([VW, HD], mybir.dt.float32r, tag="ones32",
                              name="ones32")
        ao_sb = persist.tile([128, 8 * SC], FP8, tag="ao", name="ao_sb")
        part_sb = [persist.tile([128, 512], BF16, tag=f"part{g}",
                                name=f"part{g}") for g in range(8)]
        qkstage = [persist.tile([128, 2 * S], FP8, tag=f"qkst{b}",
                                name=f"qkst{b}") for b in range(B)]
        bstage = [persist.tile([2, S], FP8, tag=f"bst{b}", name=f"bst{b}")
                  for b in range(B)]

        # ---------------- loads (ordered by first use) ----------------
        def xt_chunk_ap(b, cq):
            return bass.AP(tensor=xt.tensor, offset=b * S + cq * 512,
                           ap=[[SS, 128], [128 * SS, 8], [1, 512]])

        wqk_src = bass.AP(tensor=wqk.tensor, offset=0,
                          ap=[[2 * FPC, 128], [128 * 2 * FPC, 8], [1, 2 * FPC]])
        nc.sync.dma_start(
            out=wqk_sb.rearrange("p (i f) -> p i f", i=8), in_=wqk_src)
        for cq in range(2):
            nc.sync.dma_start(
                out=xt_sb[0][cq].rearrange("p (i s) -> p i s", i=8),
                in_=xt_chunk_ap(0, cq))
        nc.sync.dma_start(out=bq_sb, in_=bq128[:, :])
        for cq in range(2, 4):
            nc.sync.dma_start(
                out=xt_sb[0][cq].rearrange("p (i s) -> p i s", i=8),
                in_=xt_chunk_ap(0, cq))
        wv_src = bass.AP(tensor=wv.tensor, offset=0,
                         ap=[[FPC, 128], [128 * FPC, 8], [1, FPC]])
        nc.sync.dma_start(
            out=wv_sb.rearrange("p (i f) -> p i f", i=8), in_=wv_src)

        def emit_late_loads():
            for cq in range(4):
                nc.sync.dma_start(
                    out=xt_sb[1][cq].rearrange("p (i s) -> p i s", i=8),
                    in_=xt_chunk_ap(1, cq))

        def emit_wout_loads():
            wout_src = bass.AP(tensor=wout.tensor, offset=0,
                               ap=[[D, 128], [128 * D, 8], [1, D]])
            nc.sync.dma_start(
                out=wout_sb.rearrange("p (i f) -> p i f", i=8), in_=wout_src)
            nc.sync.dma_start(out=bout_sb, in_=bout8[:, :])
            nc.sync.dma_start(out=ones32, in_=onesr[:, :])

        a2a_in = [dram.tile([8, HD, SC], FP8, tag=f"a2a_in{h}",
                            name=f"a2a_in{h}", bufs=1) for h in range(HPC)]
        a2a_out = [dram.tile([8, HD, SC], FP8, tag=f"a2a_out{h}",
                             name=f"a2a_out{h}", bufs=1) for h in range(HPC)]

        def emit_a2a(h):
            if with_collective:
                nc.gpsimd.collective_compute(
                    "AllToAll", mybir.AluOpType.bypass,
                    replica_groups=[list(range(8))],
                    ins=[a2a_in[h][:, :, :].opt()],
                    outs=[a2a_out[h][:, :, :].opt()])

        # rearranged views for DoubleRow pair slicing
        xt_r = [[xt_sb[b][cq].rearrange("p (i s) -> p i s", i=8)
                 for cq in range(4)] for b in range(B)]
        wqk_r = wqk_sb.rearrange("p (i f) -> p i f", i=8)
        wv_r = wv_sb.rearrange("p (i f) -> p i f", i=8)
        wout_r = wout_sb.rearrange("p (i f) -> p i f", i=8)
        q2r = [[q2s[b][h].rearrange("p (t s) -> p t s", t=2)
                for h in range(HPC)] for b in range(B)]
        k2r = [[k2s[b][h].rearrange("p (t s) -> p t s", t=2)
                for h in range(HPC)] for b in range(B)]
        v_r = [v_all[b].rearrange("p (kb h w) -> p kb h w", kb=NKB, h=HPC)
               for b in range(B)]

        cast_cnt = [0]

        def cast_eng():
            cast_cnt[0] += 1
            return nc.scalar if cast_cnt[0] % 2 else None


        # ------------- projections -------------
        def emit_qk(b, qn):
            # q and k into one [128,1024] psum -> single strided fp8 cast
            ps = pps.tile([128, 1024], F32, tag="scores", name="ps_qk")
            for m in range(2):
                for kp in range(4):
                    nc.tensor.matmul(
                        ps[:, m * 512:(m + 1) * 512],
                        wqk_r[:, 2 * kp:2 * kp + 2,
                              m * 128:(m + 1) * 128],
                        xt_r[b][qn][:, 2 * kp:2 * kp + 2, :],
                        start=(kp == 0), stop=(kp == 3), perf_mode=DRM)
            qk8 = qkstage[b].rearrange("p (m s) -> p m s", m=2)[
                :, :, qn * 512:(qn + 1) * 512]
            psr = ps.rearrange("p (m s) -> p m s", m=2)
            if cast_eng():
                nc.scalar.copy(qk8, psr)
            else:
                nc.vector.tensor_copy(qk8, psr)
            # bias row: (bq/4 . k) per head -> fp8 staging
            k8 = qkstage[b][:, S + qn * 512:S + (qn + 1) * 512]
            bps = pps.tile([2, 512], F32, tag="scores", name="bps")
            nc.tensor.matmul(bps, bq_sb, k8, start=True, stop=True)
            b8 = bstage[b][:, qn * 512:(qn + 1) * 512]
            if cast_eng():
                nc.scalar.copy(b8, bps)
            else:
                nc.vector.tensor_copy(b8, bps)

        def emit_qk_flush(b, half=None):
            # remap DMA per (m, h, t) + bias rows; half=0/1 flushes the
            # qn 0-1 / 2-3 column halves only (keys 0-1023 / 1024-2047)
            lo, hi = (0, S) if half is None else (half * 1024,
                                                 (half + 1) * 1024)
            for m, dst_l in ((0, q2s), (1, k2s)):
                for h in range(HPC):
                    for t in range(2):
                        nc.sync.dma_start(
                            out=dst_l[b][h][0:32, t * S + lo:t * S + hi],
                            in_=qkstage[b][h * 64 + t * 32:
                                           h * 64 + t * 32 + 32,
                                           m * S + lo:m * S + hi])
            for h in range(HPC):
                nc.sync.dma_start(out=k2s[b][h][32:33, lo:hi],
                                  in_=bstage[b][h:h + 1, lo:hi])

        def emit_v(b, cq):
            ps = pps.tile([128, 512], F32, tag="scores", name="ps_v")
            for kp in range(4):
                for t in range(4):
                    nc.tensor.matmul(
                        ps[:, t * 128:(t + 1) * 128],
                        xt_r[b][cq][:, 2 * kp:2 * kp + 2,
                                    t * 128:(t + 1) * 128],
                        wv_r[:, 2 * kp:2 * kp + 2, :],
                        start=(kp == 0), stop=(kp == 3), perf_mode=DRM)
            dst = v_r[b][:, cq * 4:(cq + 1) * 4, :, 0:HD]
            src = ps.rearrange("p (t h w) -> p t h w", t=4, h=HPC)
            if cast_eng():
                nc.scalar.copy(dst, src)
            else:
                nc.vector.tensor_copy(dst, src)

        # ------------- attention -------------
        def emit_exp(b, h, kb, ps_s, dst):
            if EXP_ENG[kb] == "A":
                nc.scalar.activation(dst, ps_s, Exp, scale=EXPSC)
            else:
                with nc.allow_low_precision(
                        reason="softmax exp via fp8 exp2 bit trick"):
                    nc.vector.tensor_scalar(
                        out=dst.bitcast(I8), in0=ps_s,
                        scalar1=A_TRICK, scalar2=B_TRICK,
                        op0=mybir.AluOpType.mult, op1=mybir.AluOpType.add)

        def emit_attn(b, h, qh, fillers=()):
            fillers = [e if isinstance(e, tuple) else (0, e)
                       for e in fillers]
            ps_o = pav.tile([VW, 1024], F32, tag="pso", name="ps_o")

            def emit_av(pair, exr):
                for q2 in range(2):
                    nc.tensor.matmul(
                        ps_o[:, q2 * 512:(q2 + 1) * 512],
                        v_r[b][:, 2 * pair:2 * pair + 2, h:h + 1, 0:VW],
                        exr[:, :, q2 * 512:(q2 + 1) * 512],
                        start=(pair == 0), stop=(pair == 7), perf_mode=DRM)

            prev = None
            for pair in range(8):
                while fillers and fillers[0][0] <= pair:
                    fillers.pop(0)[1]()
                ex = pexp.tile([128, 2048], FP8, tag="ex", name="ex")
                for j in range(2):
                    kb = 2 * pair + j
                    ps_s = pscore.tile([128, 1024], F32, tag="scores",
                                       name="ps_s")
                    for q2 in range(2):
                        nc.tensor.matmul(
                            ps_s[:, q2 * 512:(q2 + 1) * 512],
                            k2r[b][h][:, :, kb * 128:(kb + 1) * 128],
                            q2r[b][h][:, :, (qh * 2 + q2) * 512:
                                      (qh * 2 + q2 + 1) * 512],
                            start=True, stop=True, perf_mode=DRM)
                    emit_exp(b, h, kb, ps_s,
                             ex[:, j * 1024:(j + 1) * 1024])
                if prev is not None:
                    emit_av(*prev)
                prev = (pair, ex.rearrange("p (j q) -> p j q", j=2))
            emit_av(*prev)
            for _, f in fillers:
                f()
            # normalization: recip emitted eagerly (DVE, doesn't block PE);
            # the bc-broadcast + multiply are returned as a closure and run
            # as a filler inside the NEXT group so the PE queue never stalls
            # on the reciprocal.
            rec = pwork.tile([VW, 1024], mybir.dt.float32r, tag="rec",
                             name="rec")
            with nc.allow_low_precision(
                    reason="softmax denom recip rounded to f32r"):
                nc.vector.reciprocal(rec[HD:VW, :], ps_o[HD:VW, :])

            def finish_q2(q2):
                bc = pps.tile([HD, 512], F32, tag="scores", name="bc")
                nc.tensor.matmul(
                    bc, ones32[HD:VW, :],
                    rec[HD:VW, q2 * 512:(q2 + 1) * 512],
                    start=True, stop=True)
                nc.vector.tensor_mul(
                    att_sb[b][h][:, (qh * 2 + q2) * 512:
                                 (qh * 2 + q2 + 1) * 512],
                    ps_o[0:HD, q2 * 512:(q2 + 1) * 512], bc)
            return (lambda: finish_q2(0)), (lambda: finish_q2(1))

        def emit_ship(b, h, half=None):
            j0, j1 = (0, 4) if half is None else (2 * half, 2 * half + 2)
            nc.sync.dma_start(
                out=a2a_in[h][b * 4 + j0:b * 4 + j1, :, :].rearrange(
                    "j p s -> p j s"),
                in_=att_sb[b][h].rearrange(
                    "p (j s) -> p j s", j=4)[:, j0:j1, :])

        srcb = a2a_out if with_collective else a2a_in

        def emit_ao_load(h):
            src4 = srcb[h].rearrange("(j two) p s -> two p j s", two=2)
            nc.sync.dma_start(
                out=ao_sb[0:HD, 4 * h * SC:(4 * h + 4) * SC].rearrange(
                    "p (j s) -> p j s", j=4),
                in_=src4[0:1, :, :, :])
            nc.sync.dma_start(
                out=ao_sb[HD:128, 4 * h * SC:(4 * h + 4) * SC].rearrange(
                    "p (j s) -> p j s", j=4),
                in_=src4[1:2, :, :, :])

        ao_r = ao_sb.rearrange("p (i s) -> p i s", i=8)

        def emit_out1(g):
            sm, en = g // 2, g % 2
            ps = pps.tile([128, 512], F32, tag="scores", name="ps_out")
            for kp in range(2):
                nc.tensor.matmul(
                    ps,
                    ao_r[:, 2 * kp:2 * kp + 2, sm * 128:(sm + 1) * 128],
                    wout_r[:, 2 * kp:2 * kp + 2, en * 512:(en + 1) * 512],
                    start=(kp == 0), stop=(kp == 1), perf_mode=DRM)
            if cast_eng():
                nc.scalar.copy(part_sb[g], ps)
            else:
                nc.vector.tensor_copy(part_sb[g], ps)

        def emit_out2(g):
            sm, en = g // 2, g % 2
            ps = pps.tile([128, 512], F32, tag="scores", name="ps_out")
            for kp in range(2, 4):
                nc.tensor.matmul(
                    ps,
                    ao_r[:, 2 * kp:2 * kp + 2, sm * 128:(sm + 1) * 128],
                    wout_r[:, 2 * kp:2 * kp + 2, en * 512:(en + 1) * 512],
                    start=(kp == 2), stop=False, perf_mode=DRM)
            nc.tensor.matmul(
                ps, ones16,
                bout_sb[:, en * 512:(en + 1) * 512],
                start=False, stop=True)
            osb = pwork.tile([128, 512], F32, tag="osb", name="osb")
            nc.vector.tensor_add(osb, ps, part_sb[g])
            nc.sync.dma_start(
                out=out[sm * 128:(sm + 1) * 128, en * 512:(en + 1) * 512],
                in_=osb)

        def F(fn, *a):
            return lambda: fn(*a)

        # ------------- schedule -------------
        for qn in range(NQC):
            emit_qk(0, qn)
        emit_qk_flush(0)
        for cq in range(4):
            emit_v(0, cq)
        emit_late_loads()
        n00 = emit_attn(0, 0, 0)
        for qn in range(NQC):
            emit_qk(1, qn)
        emit_qk_flush(1)
        emit_wout_loads()
        n01 = emit_attn(0, 1, 0, fillers=[(2, n00[0]), (4, n00[1])])
        for cq in range(4):
            emit_v(1, cq)
        n02 = emit_attn(0, 0, 1, fillers=[(2, n01[0]), (4, n01[1])])
        n03 = emit_attn(0, 1, 1, fillers=[(2, n02[0]), (4, n02[1]),
                                          (3, F(emit_ship, 0, 0))])
        n10 = emit_attn(1, 0, 0, fillers=[(2, n03[0]), (4, n03[1]),
                                          (3, F(emit_ship, 0, 1))])
        n11 = emit_attn(1, 0, 1, fillers=[(2, n10[0]), (4, n10[1])])
        n12 = emit_attn(1, 1, 0, fillers=[(2, n11[0]), (4, n11[1]),
                                          (3, F(emit_ship, 1, 0)),
                                          (4, F(emit_a2a, 0)),
                                          (5, F(emit_ao_load, 0))])
        n13 = emit_attn(1, 1, 1, fillers=[(2, n12[0]), (4, n12[1]),
                                          (4, F(emit_out1, 0)),
                                          (5, F(emit_out1, 1)),
                                          (6, F(emit_ship, 1, 1, 0)),
                                          (6, F(emit_out1, 2)),
                                          (7, F(emit_out1, 3))])
        n13[0]()
        n13[1]()
        emit_ship(1, 1, 1)
        for g in range(4, 8):
            emit_out1(g)
        emit_a2a(1)
        emit_ao_load(1)
        for g in range(8):
            emit_out2(g)

    nc.compile()
    return nc


_NC_CACHE = {}


def _get_nc(with_collective: bool = True):
    key = bool(with_collective)
    if key not in _NC_CACHE:
        _NC_CACHE[key] = _build_nc(with_collective)
    return _NC_CACHE[key]


def make_in_maps(x, w_qkv, b_qkv, w_out, b_out):
    """Host-side sharding/prep. Returns per-core input dicts."""
    x = np.asarray(x, dtype=np.float32)
    w_qkv = np.asarray(w_qkv, dtype=np.float32)
    b_qkv = np.asarray(b_qkv, dtype=np.float32)
    w_out = np.asarray(w_out, dtype=np.float32)
    b_out = np.asarray(b_out, dtype=np.float32)

    wq = w_qkv[0:D].reshape(H, HD, D)
    wk = w_qkv[D:2 * D].reshape(H, HD, D)
    wv_ = w_qkv[2 * D:3 * D].reshape(H, HD, D)
    bq = b_qkv[0:D].reshape(H, HD)
    bv = b_qkv[2 * D:3 * D]
    b_eff = b_out + w_out @ bv

    perm = np.concatenate(
        [np.arange(h * HD, (h + 1) * HD) for h in range(0, H, 2)]
        + [np.arange(h * HD, (h + 1) * HD) for h in range(1, H, 2)])
    wout_t = np.ascontiguousarray(w_out.T[perm]).astype(NPFP8)
    bout_t = np.tile((b_eff * 2.0).astype(NPFP8)[None, :], (8, 1))

    xt_all = np.ascontiguousarray(
        np.concatenate([x[0].T, x[1].T], axis=1)).astype(NPFP8)

    in_maps = []
    for c in range(NCORES):
        hs = slice(c * HPC, (c + 1) * HPC)
        wq_c = (wq[hs].reshape(FPC, D) * 32.0).T
        wk_c = (wk[hs].reshape(FPC, D) * 32.0).T
        wqk_c = np.concatenate([wq_c, wk_c], axis=1).astype(NPFP8)
        # bq128: col h = bq*8 on rows h*64..h*64+64, else 0
        bq_c = np.zeros((128, 2), dtype=np.float32)
        for h in range(HPC):
            bq_c[h * 64:(h + 1) * 64, h] = bq[c * HPC + h] * 8.0
        wv_c = (wv_[hs].reshape(FPC, D) * 32.0).T.astype(NPFP8)
        in_maps.append({
            "onesr": np.full((VW, HD), 1.0 / 32.0, dtype=np.float32),
            "xt": xt_all,
            "wqk": np.ascontiguousarray(wqk_c),
            "bq128": bq_c.astype(NPFP8),
            "wv": np.ascontiguousarray(wv_c),
            "wout": wout_t,
            "bout8": bout_t,
        })
    return in_maps


def assemble_output(results):
    out = np.empty((B, S, D), dtype=np.float32)
    for c in range(NCORES):
        b, sg = c // 4, c % 4
        out[b, sg * SC:(sg + 1) * SC, :] = results[c]["out"]
    return out


def kernel(x, mask, w_qkv, b_qkv, w_out, b_out):
    nc = _get_nc(True)
    in_maps = make_in_maps(x, w_qkv, b_qkv, w_out, b_out)
    res = run_bass_kernel_spmd(nc, in_maps, core_ids=list(range(NCORES)))
    return assemble_output(res.results)


# revision 17
# speedup vs baseline: 1.0439x; 1.0098x over previous
"""Multi-head attention on 8 Trainium2 NeuronCores (Bass/Tile), fp8 edition.

Problem: x[2,2048,1024] -> qkv proj (16 heads, hd=64) -> softmax(QK^T/8)V
-> out proj.  mask is all-ones (per spec) and is ignored.

Sharding: core c owns heads {2c, 2c+1} for BOTH batches (tensor-parallel
QKV + attention).  An 8-core AllToAll converts the head-sharded attention
output into a sequence-sharded full-feature activation; core c ends up
with global row chunk c (batch c//4, rows (c%4)*512..) and computes the
output projection full-width.

All matmuls run in fp8e4m3 with DoubleRow perf mode (2 K-subtiles per
pass).  Scales: weights x32 into fp8 range; scores accumulate f32 as
8192*score_true; exp applies 1/8192.  Biases:
  - k bias dropped (softmax-invariant along keys),
  - q bias folded into scores via an augmented 33rd contraction row
    (k'' row 32 holds fp8(bq.k)/4, q'' row 32 holds 4.0),
  - v bias folded into the output-projection bias (b_eff = b_out + W_o@b_v),
  - b_eff enters the out-proj PSUM via an fp8 ones-chunk matmul.
exp is computed per key-block [128,1024] and split between the Act engine
(table Exp, fp8 out) and the DVE (bit-exact int8 exp2 trick: fp8 bit
pattern = round(score*8*log2e/8192 + 55.656), verified vs HW).

PSUM: scores [128,1024] x2 (4 banks) + AV [65,1024] (2) + work [*,512]
x2 (2) = 8 banks exactly.
"""

import numpy as np
from contextlib import ExitStack

import concourse.bass as bass
import concourse.mybir as mybir
import concourse.tile as tile
from concourse import bacc
from concourse.bass_utils import run_bass_kernel_spmd

BF16 = mybir.dt.bfloat16
F32 = mybir.dt.float32
FP8 = mybir.dt.float8e4
I8 = mybir.dt.int8
NPFP8 = mybir.dt.np(FP8)
DRM = mybir.MatmulPerfMode.DoubleRow
Exp = mybir.ActivationFunctionType.Exp

D, H, HD, B, S = 1024, 16, 64, 2, 2048
NCORES = 8
HPC = 2              # heads per core
FPC = HPC * HD       # 128 features per core
SS = B * S           # 4096 stacked sequence (batch-major)
SC = 512             # output rows per core (post all-to-all)
NKB = S // 128       # 16 key blocks per batch
NQC = S // 512       # 4 query chunks per batch
VW = HD + 1          # live v columns per head (vd + ones)
VWP = 80             # padded v block stride (16-aligned)

EXPSC = 1.0 / 8192.0
A_TRICK = 8.0 * np.log2(np.e) / 8192.0
B_TRICK = 56.0 - 0.344   # fp8 exponent offset + PWL centering

# exp engine per kb: 'A' = Act (table exp), 'D' = DVE (int8 trick)
EXP_ENG = "ADAADADAADADAADA"


def _build_nc(with_collective: bool = True):
    nc = bacc.Bacc("TRN2", target_bir_lowering=False, debug=False,
                   num_devices=NCORES)
    xt = nc.dram_tensor("xt", [D, SS], FP8, kind="ExternalInput").ap()
    wqk = nc.dram_tensor("wqk", [D, 2 * FPC], FP8, kind="ExternalInput").ap()
    bq128 = nc.dram_tensor("bq128", [128, 2], FP8, kind="ExternalInput").ap()
    wv = nc.dram_tensor("wv", [D, FPC], FP8, kind="ExternalInput").ap()
    wout = nc.dram_tensor("wout", [D, D], FP8, kind="ExternalInput").ap()
    bout8 = nc.dram_tensor("bout8", [8, D], FP8, kind="ExternalInput").ap()
    onesr = nc.dram_tensor("onesr", [VW, HD], mybir.dt.float32r,
                           kind="ExternalInput").ap()
    out = nc.dram_tensor("out", [SC, D], F32, kind="ExternalOutput").ap()

    with ExitStack() as ctx:
        tc = ctx.enter_context(tile.TileContext(nc))
        persist = ctx.enter_context(tc.tile_pool(name="persist", bufs=1))
        pexp = ctx.enter_context(tc.tile_pool(name="pexp", bufs=6))
        pwork = ctx.enter_context(tc.tile_pool(name="pwork", bufs=3))
        pscore = ctx.enter_context(tc.tile_pool(name="pscore", bufs=3,
                                                space="PSUM"))
        pav = ctx.enter_context(tc.tile_pool(name="pav", bufs=1,
                                             space="PSUM"))
        pps = pscore
        dram = ctx.enter_context(tc.tile_pool(name="dram", bufs=2,
                                              space="DRAM"))

        # ---------------- persistent SBUF ----------------
        xt_sb = [[persist.tile([128, 8 * 512], FP8, tag=f"xt{b}_{cq}",
                               name=f"xt{b}_{cq}") for cq in range(4)]
                 for b in range(B)]
        wqk_sb = persist.tile([128, 8 * 2 * FPC], FP8, tag="wqk",
                              name="wqk_sb")
        bq_sb = persist.tile([128, 2], FP8, tag="bq", name="bq_sb")
        wv_sb = persist.tile([128, 8 * FPC], FP8, tag="wv", name="wv_sb")
        wout_sb = persist.tile([128, 8 * D], FP8, tag="wout", name="wout_sb")
        bout_sb = persist.tile([8, D], FP8, tag="bout", name="bout_sb")
        ones16 = persist.tile([8, 128], FP8, tag="ones16", name="ones16")
        nc.gpsimd.memset(ones16, 0.0625)

        # q2s/k2s[b][h]: [33, 2*2048] — hd-half t at cols t*2048;
        # row 32: q'' ones (4.0 at t0) / k'' bias row (device-filled)
        q2s = [[persist.tile([33, 2 * S], FP8, tag=f"q2_{b}_{h}",
                             name=f"q2_{b}_{h}") for h in range(HPC)]
               for b in range(B)]
        k2s = [[persist.tile([33, 2 * S], FP8, tag=f"k2_{b}_{h}",
                             name=f"k2_{b}_{h}") for h in range(HPC)]
               for b in range(B)]
        for b in range(B):
            for h in range(HPC):
                nc.gpsimd.memset(q2s[b][h][32:33, 0:S], 4.0)
                nc.gpsimd.memset(q2s[b][h][32:33, S:2 * S], 0.0)
                nc.gpsimd.memset(k2s[b][h][32:33, S:2 * S], 0.0)

        # v_all[b]: [128, kb(16) x h(2) x 80]; col 64 = ones
        v_all = [persist.tile([128, NKB * HPC * VWP], FP8, tag=f"v{b}",
                              name=f"v{b}") for b in range(B)]
        for b in range(B):
            vr = v_all[b].rearrange("p (kb h w) -> p kb h w", kb=NKB, h=HPC)
            nc.gpsimd.memset(vr[:, :, :, HD:VW], 1.0)

        att_sb = [[persist.tile([64, S], FP8, tag=f"att{b}_{h}",
                                name=f"att{b}_{h}") for h in range(HPC)]
                  for b in range(B)]
        ones32 = persist.tile([VW, HD], mybir.dt.float32r, tag="ones32",
                              name="ones32")
        ao_sb = persist.tile([128, 8 * SC], FP8, tag="ao", name="ao_sb")
        part_sb = [persist.tile([128, 512], BF16, tag=f"part{g}",
                                name=f"part{g}") for g in range(8)]
        qkstage = [persist.tile([128, 2 * S], FP8, tag=f"qkst{b}",
                                name=f"qkst{b}") for b in range(B)]
        bstage = [persist.tile([2, S], FP8, tag=f"bst{b}", name=f"bst{b}")
                  for b in range(B)]

        # ---------------- loads (ordered by first use) ----------------
        def xt_chunk_ap(b, cq):
            return bass.AP(tensor=xt.tensor, offset=b * S + cq * 512,
                           ap=[[SS, 128], [128 * SS, 8], [1, 512]])

        wqk_src = bass.AP(tensor=wqk.tensor, offset=0,
                          ap=[[2 * FPC, 128], [128 * 2 * FPC, 8], [1, 2 * FPC]])
        nc.sync.dma_start(
            out=wqk_sb.rearrange("p (i f) -> p i f", i=8), in_=wqk_src)
        for cq in range(2):
            nc.sync.dma_start(
                out=xt_sb[0][cq].rearrange("p (i s) -> p i s", i=8),
                in_=xt_chunk_ap(0, cq))
        nc.sync.dma_start(out=bq_sb, in_=bq128[:, :])
        for cq in range(2, 4):
            nc.sync.dma_start(
                out=xt_sb[0][cq].rearrange("p (i s) -> p i s", i=8),
                in_=xt_chunk_ap(0, cq))
        wv_src = bass.AP(tensor=wv.tensor, offset=0,
                         ap=[[FPC, 128], [128 * FPC, 8], [1, FPC]])
        nc.sync.dma_start(
            out=wv_sb.rearrange("p (i f) -> p i f", i=8), in_=wv_src)

        def emit_late_loads():
            for cq in range(4):
                nc.sync.dma_start(
                    out=xt_sb[1][cq].rearrange("p (i s) -> p i s", i=8),
                    in_=xt_chunk_ap(1, cq))

        def emit_wout_loads():
            wout_src = bass.AP(tensor=wout.tensor, offset=0,
                               ap=[[D, 128], [128 * D, 8], [1, D]])
            nc.sync.dma_start(
                out=wout_sb.rearrange("p (i f) -> p i f", i=8), in_=wout_src)
            nc.sync.dma_start(out=bout_sb, in_=bout8[:, :])
            nc.sync.dma_start(out=ones32, in_=onesr[:, :])

        a2a_in = [dram.tile([8, HD, SC], FP8, tag=f"a2a_in{h}",
                            name=f"a2a_in{h}", bufs=1) for h in range(HPC)]
        a2a_out = [dram.tile([8, HD, SC], FP8, tag=f"a2a_out{h}",
                             name=f"a2a_out{h}", bufs=1) for h in range(HPC)]

        def emit_a2a(h):
            if with_collective:
                nc.gpsimd.collective_compute(
                    "AllToAll", mybir.AluOpType.bypass,
                    replica_groups=[list(range(8))],
                    ins=[a2a_in[h][:, :, :].opt()],
                    outs=[a2a_out[h][:, :, :].opt()])

        # rearranged views for DoubleRow pair slicing
        xt_r = [[xt_sb[b][cq].rearrange("p (i s) -> p i s", i=8)
                 for cq in range(4)] for b in range(B)]
        wqk_r = wqk_sb.rearrange("p (i f) -> p i f", i=8)
        wv_r = wv_sb.rearrange("p (i f) -> p i f", i=8)
        wout_r = wout_sb.rearrange("p (i f) -> p i f", i=8)
        q2r = [[q2s[b][h].rearrange("p (t s) -> p t s", t=2)
                for h in range(HPC)] for b in range(B)]
        k2r = [[k2s[b][h].rearrange("p (t s) -> p t s", t=2)
                for h in range(HPC)] for b in range(B)]
        v_r = [v_all[b].rearrange("p (kb h w) -> p kb h w", kb=NKB, h=HPC)
               for b in range(B)]

        cast_cnt = [0]

        def cast_eng():
            cast_cnt[0] += 1
            return nc.scalar if cast_cnt[0] % 2 else None


        # ------------- projections -------------
        def emit_qk(b, qn):
            # q and k into one [128,1024] psum -> single strided fp8 cast
            ps = pps.tile([128, 1024], F32, tag="scores", name="ps_qk")
            for m in range(2):
                for kp in range(4):
                    nc.tensor.matmul(
                        ps[:, m * 512:(m + 1) * 512],
                        wqk_r[:, 2 * kp:2 * kp + 2,
                              m * 128:(m + 1) * 128],
                        xt_r[b][qn][:, 2 * kp:2 * kp + 2, :],
                        start=(kp == 0), stop=(kp == 3), perf_mode=DRM)
            qk8 = qkstage[b].rearrange("p (m s) -> p m s", m=2)[
                :, :, qn * 512:(qn + 1) * 512]
            psr = ps.rearrange("p (m s) -> p m s", m=2)
            if cast_eng():
                nc.scalar.copy(qk8, psr)
            else:
                nc.vector.tensor_copy(qk8, psr)
            # bias row: (bq/4 . k) per head -> fp8 staging
            k8 = qkstage[b][:, S + qn * 512:S + (qn + 1) * 512]
            bps = pps.tile([2, 512], F32, tag="scores", name="bps")
            nc.tensor.matmul(bps, bq_sb, k8, start=True, stop=True)
            b8 = bstage[b][:, qn * 512:(qn + 1) * 512]
            if cast_eng():
                nc.scalar.copy(b8, bps)
            else:
                nc.vector.tensor_copy(b8, bps)

        def emit_qk_flush(b, half=None):
            # remap DMA per (m, h, t) + bias rows; half=0/1 flushes the
            # qn 0-1 / 2-3 column halves only (keys 0-1023 / 1024-2047)
            lo, hi = (0, S) if half is None else (half * 1024,
                                                 (half + 1) * 1024)
            for m, dst_l in ((0, q2s), (1, k2s)):
                for h in range(HPC):
                    for t in range(2):
                        nc.sync.dma_start(
                            out=dst_l[b][h][0:32, t * S + lo:t * S + hi],
                            in_=qkstage[b][h * 64 + t * 32:
                                           h * 64 + t * 32 + 32,
                                           m * S + lo:m * S + hi])
            for h in range(HPC):
                nc.sync.dma_start(out=k2s[b][h][32:33, lo:hi],
                                  in_=bstage[b][h:h + 1, lo:hi])

        def emit_v(b, cq):
            ps = pps.tile([128, 512], F32, tag="scores", name="ps_v")
            for kp in range(4):
                for t in range(4):
                    nc.tensor.matmul(
                        ps[:, t * 128:(t + 1) * 128],
                        xt_r[b][cq][:, 2 * kp:2 * kp + 2,
                                    t * 128:(t + 1) * 128],
                        wv_r[:, 2 * kp:2 * kp + 2, :],
                        start=(kp == 0), stop=(kp == 3), perf_mode=DRM)
            dst = v_r[b][:, cq * 4:(cq + 1) * 4, :, 0:HD]
            src = ps.rearrange("p (t h w) -> p t h w", t=4, h=HPC)
            if cast_eng():
                nc.scalar.copy(dst, src)
            else:
                nc.vector.tensor_copy(dst, src)

        # ------------- attention -------------
        def emit_exp(b, h, kb, ps_s, dst):
            if EXP_ENG[kb] == "A":
                nc.scalar.activation(dst, ps_s, Exp, scale=EXPSC)
            else:
                with nc.allow_low_precision(
                        reason="softmax exp via fp8 exp2 bit trick"):
                    nc.vector.tensor_scalar(
                        out=dst.bitcast(I8), in0=ps_s,
                        scalar1=A_TRICK, scalar2=B_TRICK,
                        op0=mybir.AluOpType.mult, op1=mybir.AluOpType.add)

        def emit_attn(b, h, qh, fillers=()):
            fillers = [e if isinstance(e, tuple) else (0, e)
                       for e in fillers]
            ps_o = pav.tile([VW, 1024], F32, tag="pso", name="ps_o")

            def emit_av(pair, exr):
                for q2 in range(2):
                    nc.tensor.matmul(
                        ps_o[:, q2 * 512:(q2 + 1) * 512],
                        v_r[b][:, 2 * pair:2 * pair + 2, h:h + 1, 0:VW],
                        exr[:, :, q2 * 512:(q2 + 1) * 512],
                        start=(pair == 0), stop=(pair == 7), perf_mode=DRM)

            prev = None
            for pair in range(8):
                while fillers and fillers[0][0] <= pair:
                    fillers.pop(0)[1]()
                ex = pexp.tile([128, 2048], FP8, tag="ex", name="ex")
                for j in range(2):
                    kb = 2 * pair + j
                    ps_s = pscore.tile([128, 1024], F32, tag="scores",
                                       name="ps_s")
                    for q2 in range(2):
                        nc.tensor.matmul(
                            ps_s[:, q2 * 512:(q2 + 1) * 512],
                            k2r[b][h][:, :, kb * 128:(kb + 1) * 128],
                            q2r[b][h][:, :, (qh * 2 + q2) * 512:
                                      (qh * 2 + q2 + 1) * 512],
                            start=True, stop=True, perf_mode=DRM)
                    emit_exp(b, h, kb, ps_s,
                             ex[:, j * 1024:(j + 1) * 1024])
                if prev is not None:
                    emit_av(*prev)
                prev = (pair, ex.rearrange("p (j q) -> p j q", j=2))
            emit_av(*prev)
            for _, f in fillers:
                f()
            # normalization: recip emitted eagerly (DVE, doesn't block PE);
            # the bc-broadcast + multiply are returned as a closure and run
            # as a filler inside the NEXT group so the PE queue never stalls
            # on the reciprocal.
            rec = pwork.tile([VW, 1024], mybir.dt.float32r, tag="rec",
                             name="rec")
            with nc.allow_low_precision(
                    reason="softmax denom recip rounded to f32r"):
                nc.vector.reciprocal(rec[HD:VW, :], ps_o[HD:VW, :])

            def finish_q2(q2):
                bc = pps.tile([HD, 512], F32, tag="scores", name="bc")
                nc.tensor.matmul(
                    bc, ones32[HD:VW, :],
                    rec[HD:VW, q2 * 512:(q2 + 1) * 512],
                    start=True, stop=True)
                nc.vector.tensor_mul(
                    att_sb[b][h][:, (qh * 2 + q2) * 512:
                                 (qh * 2 + q2 + 1) * 512],
                    ps_o[0:HD, q2 * 512:(q2 + 1) * 512], bc)
            return (lambda: finish_q2(0)), (lambda: finish_q2(1))

        def emit_ship(b, h, half=None):
            j0, j1 = (0, 4) if half is None else (2 * half, 2 * half + 2)
            nc.sync.dma_start(
                out=a2a_in[h][b * 4 + j0:b * 4 + j1, :, :].rearrange(
                    "j p s -> p j s"),
                in_=att_sb[b][h].rearrange(
                    "p (j s) -> p j s", j=4)[:, j0:j1, :])

        srcb = a2a_out if with_collective else a2a_in

        def emit_ao_load(h):
            src4 = srcb[h].rearrange("(j two) p s -> two p j s", two=2)
            nc.sync.dma_start(
                out=ao_sb[0:HD, 4 * h * SC:(4 * h + 4) * SC].rearrange(
                    "p (j s) -> p j s", j=4),
                in_=src4[0:1, :, :, :])
            nc.sync.dma_start(
                out=ao_sb[HD:128, 4 * h * SC:(4 * h + 4) * SC].rearrange(
                    "p (j s) -> p j s", j=4),
                in_=src4[1:2, :, :, :])

        ao_r = ao_sb.rearrange("p (i s) -> p i s", i=8)

        def emit_out1(g):
            sm, en = g // 2, g % 2
            ps = pps.tile([128, 512], F32, tag="scores", name="ps_out")
            for kp in range(2):
                nc.tensor.matmul(
                    ps,
                    ao_r[:, 2 * kp:2 * kp + 2, sm * 128:(sm + 1) * 128],
                    wout_r[:, 2 * kp:2 * kp + 2, en * 512:(en + 1) * 512],
                    start=(kp == 0), stop=(kp == 1), perf_mode=DRM)
            if cast_eng():
                nc.scalar.copy(part_sb[g], ps)
            else:
                nc.vector.tensor_copy(part_sb[g], ps)

        def emit_out(g):
            sm, en = g // 2, g % 2
            ps = pps.tile([128, 512], F32, tag="scores", name="ps_out")
            for kp in range(4):
                nc.tensor.matmul(
                    ps,
                    ao_r[:, 2 * kp:2 * kp + 2, sm * 128:(sm + 1) * 128],
                    wout_r[:, 2 * kp:2 * kp + 2, en * 512:(en + 1) * 512],
                    start=(kp == 0), stop=False, perf_mode=DRM)
            nc.tensor.matmul(
                ps, ones16,
                bout_sb[:, en * 512:(en + 1) * 512],
                start=False, stop=True)
            osb = pwork.tile([128, 512], F32, tag="osb", name="osb")
            if cast_eng():
                nc.scalar.copy(osb, ps)
            else:
                nc.vector.tensor_copy(osb, ps)
            nc.sync.dma_start(
                out=out[sm * 128:(sm + 1) * 128, en * 512:(en + 1) * 512],
                in_=osb)

        def emit_out2(g):
            sm, en = g // 2, g % 2
            ps = pps.tile([128, 512], F32, tag="scores", name="ps_out")
            for kp in range(2, 4):
                nc.tensor.matmul(
                    ps,
                    ao_r[:, 2 * kp:2 * kp + 2, sm * 128:(sm + 1) * 128],
                    wout_r[:, 2 * kp:2 * kp + 2, en * 512:(en + 1) * 512],
                    start=(kp == 2), stop=False, perf_mode=DRM)
            nc.tensor.matmul(
                ps, ones16,
                bout_sb[:, en * 512:(en + 1) * 512],
                start=False, stop=True)
            osb = pwork.tile([128, 512], F32, tag="osb", name="osb")
            nc.vector.tensor_add(osb, ps, part_sb[g])
            nc.sync.dma_start(
                out=out[sm * 128:(sm + 1) * 128, en * 512:(en + 1) * 512],
                in_=osb)

        def F(fn, *a):
            return lambda: fn(*a)

        # ------------- schedule -------------
        for qn in range(NQC):
            emit_qk(0, qn)
        emit_qk_flush(0)
        for cq in range(4):
            emit_v(0, cq)
        emit_late_loads()
        n00 = emit_attn(0, 0, 0)
        for qn in range(NQC):
            emit_qk(1, qn)
        emit_qk_flush(1)
        emit_wout_loads()
        n01 = emit_attn(0, 1, 0, fillers=[(2, n00[0]), (4, n00[1])])
        for cq in range(4):
            emit_v(1, cq)
        n02 = emit_attn(0, 0, 1, fillers=[(2, n01[0]), (4, n01[1])])
        n03 = emit_attn(0, 1, 1, fillers=[(2, n02[0]), (4, n02[1]),
                                          (3, F(emit_ship, 0, 0))])
        n10 = emit_attn(1, 0, 0, fillers=[(2, n03[0]), (4, n03[1]),
                                          (3, F(emit_ship, 0, 1))])
        n11 = emit_attn(1, 0, 1, fillers=[(2, n10[0]), (4, n10[1])])
        n12 = emit_attn(1, 1, 0, fillers=[(2, n11[0]), (4, n11[1]),
                                          (3, F(emit_ship, 1, 0)),
                                          (4, F(emit_a2a, 0)),
                                          (5, F(emit_ao_load, 0))])
        n13 = emit_attn(1, 1, 1, fillers=[(2, n12[0]), (4, n12[1])])
        n13[0]()
        n13[1]()
        emit_ship(1, 1)
        emit_a2a(1)
        emit_ao_load(1)
        for g in range(8):
            emit_out(g)

    nc.compile()
    return nc


_NC_CACHE = {}


def _get_nc(with_collective: bool = True):
    key = bool(with_collective)
    if key not in _NC_CACHE:
        _NC_CACHE[key] = _build_nc(with_collective)
    return _NC_CACHE[key]


def make_in_maps(x, w_qkv, b_qkv, w_out, b_out):
    """Host-side sharding/prep. Returns per-core input dicts."""
    x = np.asarray(x, dtype=np.float32)
    w_qkv = np.asarray(w_qkv, dtype=np.float32)
    b_qkv = np.asarray(b_qkv, dtype=np.float32)
    w_out = np.asarray(w_out, dtype=np.float32)
    b_out = np.asarray(b_out, dtype=np.float32)

    wq = w_qkv[0:D].reshape(H, HD, D)
    wk = w_qkv[D:2 * D].reshape(H, HD, D)
    wv_ = w_qkv[2 * D:3 * D].reshape(H, HD, D)
    bq = b_qkv[0:D].reshape(H, HD)
    bv = b_qkv[2 * D:3 * D]
    b_eff = b_out + w_out @ bv

    perm = np.concatenate(
        [np.arange(h * HD, (h + 1) * HD) for h in range(0, H, 2)]
        + [np.arange(h * HD, (h + 1) * HD) for h in range(1, H, 2)])
    wout_t = np.ascontiguousarray(w_out.T[perm]).astype(NPFP8)
    bout_t = np.tile((b_eff * 2.0).astype(NPFP8)[None, :], (8, 1))

    xt_all = np.ascontiguousarray(
        np.concatenate([x[0].T, x[1].T], axis=1)).astype(NPFP8)

    in_maps = []
    for c in range(NCORES):
        hs = slice(c * HPC, (c + 1) * HPC)
        wq_c = (wq[hs].reshape(FPC, D) * 32.0).T
        wk_c = (wk[hs].reshape(FPC, D) * 32.0).T
        wqk_c = np.concatenate([wq_c, wk_c], axis=1).astype(NPFP8)
        # bq128: col h = bq*8 on rows h*64..h*64+64, else 0
        bq_c = np.zeros((128, 2), dtype=np.float32)
        for h in range(HPC):
            bq_c[h * 64:(h + 1) * 64, h] = bq[c * HPC + h] * 8.0
        wv_c = (wv_[hs].reshape(FPC, D) * 32.0).T.astype(NPFP8)
        in_maps.append({
            "onesr": np.full((VW, HD), 1.0 / 32.0, dtype=np.float32),
            "xt": xt_all,
            "wqk": np.ascontiguousarray(wqk_c),
            "bq128": bq_c.astype(NPFP8),
            "wv": np.ascontiguousarray(wv_c),
            "wout": wout_t,
            "bout8": bout_t,
        })
    return in_maps


def assemble_output(results):
    out = np.empty((B, S, D), dtype=np.float32)
    for c in range(NCORES):
        b, sg = c // 4, c % 4
        out[b, sg * SC:(sg + 1) * SC, :] = results[c]["out"]
    return out


def kernel(x, mask, w_qkv, b_qkv, w_out, b_out):
    nc = _get_nc(True)
    in_maps = make_in_maps(x, w_qkv, b_qkv, w_out, b_out)
    res = run_bass_kernel_spmd(nc, in_maps, core_ids=list(range(NCORES)))
    return assemble_output(res.results)
